# revision 13
# baseline (speedup 1.0000x reference)
# Differential multi-head attention (dual softmax + GroupNorm + sigmoid gating)
# for Trainium2, batch-parallel across 8 NeuronCores (one batch row per core).
#
# Per-core math (batch b):
#   q = query @ Wq + bq -> per head: q1, q2, gate (each S x 64)
#   k = key   @ Wk + bk -> per head: k1, k2
#   v = values@ Wv + bv -> per head: v (S x 64)
#   attn = softmax(q1 k1^T / 8) - lam * softmax(q2 k2^T / 8)
#   out  = GroupNorm_{8 groups over d, reduced over (S, heads, d-in-group)}(attn @ v)
#   out  = out * (1 - lambda_init) * sigmoid(gate)
#
# Layout strategy: d-major ("transposed") attention: scores are computed as
# s^T (k on partitions, q free) so the attn@v contraction runs at K=128, and
# exp row-sums come free via a ones-column appended to v (M=65).
#
# Engine budget: ACT (scalar) is the bottleneck -- 128 exp tiles of
# [128,1024] at (N+352)/1.2GHz ~= 147us is the floor.  Everything else is
# kept off ACT: weight downcasts + x casts on Pool (gpsimd), projection
# epilogues + va/os copies + combines on DVE, gate tanh + output PSUM
# copies in the tail where ACT is idle.  Score matmuls are K=64 row-split
# (term1 rows 0-63, term2 rows 64-127, interleaved per chunk) so the PE
# streams both terms concurrently via row tiling.

import numpy as np

B, S_FULL, H, D = 8, 1024, 8, 64
DM = H * D  # 512


def build_nc(S=1024):
    import concourse.bacc as bacc
    import concourse.bass as bass
    import concourse.tile as tile
    from concourse import mybir
    from concourse.masks import make_identity

    f32 = mybir.dt.float32
    bf16 = mybir.dt.bfloat16
    AF = mybir.ActivationFunctionType
    OP = mybir.AluOpType
    AX = mybir.AxisListType

    NJ = S // 128          # k/seq 128-tiles
    CH = min(512, S)       # fp32-out matmul chunk
    NN = max(1, S // CH)
    CNT = float(S * H * (D // H))  # groupnorm reduction count per group
    EPS = 1e-3
    INV = 0.125            # 1/sqrt(64)

    nc = bacc.Bacc(target_bir_lowering=False)
    q_d = nc.dram_tensor("query", [S, DM], f32, kind="ExternalInput")
    k_d = nc.dram_tensor("key", [S, DM], f32, kind="ExternalInput")
    v_d = nc.dram_tensor("values", [S, DM], f32, kind="ExternalInput")
    wq_d = nc.dram_tensor("Wq", [DM, 3 * H * D], f32, kind="ExternalInput")
    bq_d = nc.dram_tensor("bq", [3 * H * D], f32, kind="ExternalInput")
    wk_d = nc.dram_tensor("Wk", [DM, 2 * H * D], f32, kind="ExternalInput")
    bk_d = nc.dram_tensor("bk", [2 * H * D], f32, kind="ExternalInput")
    wv_d = nc.dram_tensor("Wv", [DM, H * D], f32, kind="ExternalInput")
    bv_d = nc.dram_tensor("bv", [H * D], f32, kind="ExternalInput")
    gamma_d = nc.dram_tensor("gamma", [D], f32, kind="ExternalInput")
    beta_d = nc.dram_tensor("beta", [D], f32, kind="ExternalInput")
    lam_d = nc.dram_tensor("lam", [1], f32, kind="ExternalInput")
    li_d = nc.dram_tensor("lambda_init", [1], f32, kind="ExternalInput")
    out_d = nc.dram_tensor("out", [S, DM], f32, kind="ExternalOutput")

    ts_ = nc.vector.tensor_scalar
    stt = nc.vector.scalar_tensor_tensor
    pts_ = nc.gpsimd.tensor_scalar
    pstt = nc.gpsimd.scalar_tensor_tensor

    with tile.TileContext(nc) as tc:
        with tc.tile_pool(name="consts", bufs=1) as consts, \
             tc.tile_pool(name="persist", bufs=1) as persist:

            # ---------- constants ----------
            # dummy exp to pull the ACT exp/tanh table load to t=0
            dmy = consts.tile([1, 8], f32, tag="dmy", name="dmy")
            nc.gpsimd.memset(dmy, 0.0)
            dmyo = consts.tile([1, 8], f32, tag="dmyo", name="dmyo")
            nc.scalar.activation(dmyo, dmy, AF.Exp)

            ident = consts.tile([128, 128], f32, tag="ident", name="ident")
            make_identity(nc, ident)
            ident_b = consts.tile([128, 128], bf16, tag="ident_b", name="ident_b")
            make_identity(nc, ident_b)

            # block-diagonal group matrix: IND2[d', d] = 1 iff d'//8 == d//8
            ind2 = consts.tile([64, 64], f32, tag="ind2", name="ind2")
            nc.gpsimd.memset(ind2, 1.0)
            nc.gpsimd.affine_select(
                out=ind2, in_=ind2, compare_op=OP.is_ge, fill=0.0,
                base=0, pattern=[[-8, 8], [0, 8]], channel_multiplier=1)
            nc.gpsimd.affine_select(
                out=ind2, in_=ind2, compare_op=OP.is_ge, fill=0.0,
                base=7, pattern=[[8, 8], [0, 8]], channel_multiplier=-1)

            # selector for the r-row broadcast matmul (used by the last pair)
            # sel2[p, x] = 1 iff (x - 64p) in [0, 64): row 0 -> parts 0-63,
            # row 1 -> parts 64-127
            sel2 = consts.tile([2, 128], f32, tag="sel2", name="sel2")
            nc.gpsimd.memset(sel2, 1.0)
            nc.gpsimd.affine_select(
                out=sel2, in_=sel2, compare_op=OP.is_ge, fill=0.0,
                base=0, pattern=[[1, 128]], channel_multiplier=-64)
            nc.gpsimd.affine_select(
                out=sel2, in_=sel2, compare_op=OP.is_ge, fill=0.0,
                base=63, pattern=[[-1, 128]], channel_multiplier=64)

            # scalar columns
            lam64 = consts.tile([64, 1], f32, tag="lam64", name="lam64")
            nc.gpsimd.dma_start(out=lam64, in_=lam_d[:].to_broadcast([64, 1]))
            li64 = consts.tile([64, 1], f32, tag="li64", name="li64")
            nc.gpsimd.dma_start(out=li64, in_=li_d[:].to_broadcast([64, 1]))
            neglam64 = consts.tile([64, 1], f32, tag="neglam64", name="neglam64")
            ts_(neglam64, lam64, -1.0, None, OP.mult)
            onelam64 = consts.tile([64, 1], f32, tag="onelam64", name="onelam64")
            ts_(onelam64, lam64, -1.0, 1.0, OP.mult, OP.add)   # 1 - lam
            halfli = consts.tile([64, 1], f32, tag="halfli", name="halfli")
            ts_(halfli, li64, -0.5, 0.5, OP.mult, OP.add)      # 0.5*(1-li)

            gamma_c = consts.tile([64, 1], f32, tag="gamma_c", name="gamma_c")
            nc.sync.dma_start(out=gamma_c, in_=gamma_d[:])
            beta_c = consts.tile([64, 1], f32, tag="beta_c", name="beta_c")
            nc.sync.dma_start(out=beta_c, in_=beta_d[:])
            bb64 = consts.tile([64, 1], f32, tag="bb64", name="bb64")
            ts_(bb64, beta_c, halfli, None, OP.mult)           # beta*0.5*(1-li)

            # v-bias columns per head and C = bv*(1-lam) fold
            bvc = consts.tile([64, 8], f32, tag="bvc", name="bvc")
            nc.sync.dma_start(
                out=bvc, in_=bv_d[:].rearrange("(h d) -> d h", d=64))
            cc = consts.tile([64, 8], f32, tag="cc", name="cc")
            ts_(cc, bvc, onelam64, None, OP.mult)

            # bias columns: per-head stacked [q1|q2] / [k1|k2] are contiguous
            # 128-element runs of bq/bk; gate needs a gathered layout.
            bqp = consts.tile([128, 8], f32, tag="bqp", name="bqp")
            nc.sync.dma_start(
                out=bqp,
                in_=bq_d[:].rearrange("(h blk) -> blk h", blk=192)[0:128, :])
            bkp = consts.tile([128, 8], f32, tag="bkp", name="bkp")
            nc.sync.dma_start(
                out=bkp,
                in_=bk_d[:].rearrange("(h blk) -> blk h", blk=128))
            bg = consts.tile([128, 4], f32, tag="bg", name="bg")
            bqv = bq_d[:].rearrange("(h blk) -> h blk", blk=192)
            for p in range(4):
                nc.sync.dma_start(out=bg[:, p:p + 1],
                                  in_=bqv[2 * p:2 * p + 2, 128:192])

            # persistent projection outputs (bf16, d-major)
            # qp/kp[h]: rows 0-63 = q1/k1 of head h, rows 64-127 = q2/k2
            qp = [persist.tile([128, S], bf16, tag=f"qp{h}", name=f"qp{h}") for h in range(8)]
            kp = [persist.tile([128, S], bf16, tag=f"kp{h}", name=f"kp{h}") for h in range(8)]
            # gate stays head-pair packed: gt[p] rows 0-63 = head 2p, 64-127 = 2p+1
            gt = [persist.tile([128, S], bf16, tag=f"gt{p}", name=f"gt{p}") for p in range(4)]
            va = [persist.tile([128, 8, 65], bf16, tag=f"va{i}", name=f"va{i}") for i in range(NJ)]
            ypair = [persist.tile([128, S], f32, tag=f"yp{p}", name=f"yp{p}") for p in range(4)]
            sumcol = persist.tile([64, 16], f32, tag="sumcol", name="sumcol")

            # ---------- phase 1: load + cast + transpose + project ----------
            GRP = min(4, NJ)
            with tc.tile_pool(name="xin", bufs=3) as xin_pool, \
                 tc.tile_pool(name="xtp", bufs=1) as xtp, \
                 tc.tile_pool(name="wload", bufs=2) as wpool, \
                 tc.tile_pool(name="ps_in", bufs=1, space="PSUM") as ps_in, \
                 tc.tile_pool(name="ps_proj", bufs=4, space="PSUM") as ps_proj:

                wqf = [wpool.tile([128, 3 * H * D], bf16, tag=f"wqf{r}", name=f"wqf{r}", bufs=1) for r in range(4)]
                wkf = [wpool.tile([128, 2 * H * D], bf16, tag=f"wkf{r}", name=f"wkf{r}", bufs=1) for r in range(4)]
                wvf = [wpool.tile([128, H * D], bf16, tag=f"wvf{r}", name=f"wvf{r}", bufs=1) for r in range(4)]

                # head-0 slices of Wq/Wk land first so the first score matmul
                # can fire after ~3 DMAs; the bulk columns stream in behind.
                for r in range(4):
                    ws0 = wpool.tile([128, 192], f32, tag="wsq0", name="wsq0")
                    nc.sync.dma_start(out=ws0, in_=wq_d[128 * r:128 * (r + 1), 0:192])
                    nc.gpsimd.tensor_copy(wqf[r][:, 0:192], ws0)
                for r in range(4):
                    ws0 = wpool.tile([128, 128], f32, tag="wsk0", name="wsk0")
                    nc.sync.dma_start(out=ws0, in_=wk_d[128 * r:128 * (r + 1), 0:128])
                    nc.gpsimd.tensor_copy(wkf[r][:, 0:128], ws0)

                def transpose_input(x_dram, nm):
                    xt = [xtp.tile([128, S], bf16, tag=f"xt{nm}{c}", name=f"xt{nm}{c}")
                          for c in range(4)]
                    tp_cur = [None] * 4
                    for i in range(NJ):
                        xs = xin_pool.tile([128, DM], f32, tag="xs", name="xs")
                        nc.sync.dma_start(out=xs, in_=x_dram[128 * i:128 * (i + 1), :])
                        xq = xin_pool.tile([128, DM], bf16, tag="xin", name="xin")
                        nc.gpsimd.tensor_copy(xq, xs)
                        if i % GRP == 0:
                            for c in range(4):
                                tp_cur[c] = ps_in.tile(
                                    [128, 128 * GRP], bf16, tag=f"tp{c}", name=f"tp{c}")
                        for c in range(4):
                            nc.tensor.transpose(
                                tp_cur[c][:, 128 * (i % GRP):128 * (i % GRP + 1)],
                                xq[:, 128 * c:128 * (c + 1)], ident_b)
                        if i % GRP == GRP - 1:
                            base = 128 * GRP * (i // GRP)
                            for c in range(4):
                                nc.vector.tensor_copy(
                                    xt[c][:, base:base + 128 * GRP], tp_cur[c])
                    return xt

                xtq = transpose_input(q_d, "q")
                xtk = transpose_input(k_d, "k")

                # wv + remaining wq/wk columns (casts on Pool)
                for r in range(4):
                    wsv = wpool.tile([128, H * D], f32, tag="wsv", name="wsv")
                    nc.sync.dma_start(out=wsv, in_=wv_d[128 * r:128 * (r + 1), :])
                    nc.gpsimd.tensor_copy(wvf[r], wsv)
                xtv = transpose_input(v_d, "v")
                for r in range(4):
                    wsq = wpool.tile([128, 3 * H * D - 192], f32, tag="wsqr", name="wsqr")
                    nc.sync.dma_start(out=wsq, in_=wq_d[128 * r:128 * (r + 1), 192:])
                    nc.gpsimd.tensor_copy(wqf[r][:, 192:], wsq)
                    wsk = wpool.tile([128, 2 * H * D - 128], f32, tag="wskr", name="wskr")
                    nc.sync.dma_start(out=wsk, in_=wk_d[128 * r:128 * (r + 1), 128:])
                    nc.gpsimd.tensor_copy(wkf[r][:, 128:], wsk)

                def proj_q(h):
                    for n in range(NN):
                        ps = ps_proj.tile([128, CH], f32, tag="proj", name="proj")
                        for r in range(4):
                            nc.tensor.matmul(
                                ps, wqf[r][:, 192 * h:192 * h + 128],
                                xtq[r][:, CH * n:CH * (n + 1)],
                                start=(r == 0), stop=(r == 3))
                        ts_(qp[h][:, CH * n:CH * (n + 1)], ps, bqp[:, h:h + 1],
                            None, OP.add)

                def proj_k(h):
                    for n in range(NN):
                        ps = ps_proj.tile([128, CH], f32, tag="proj", name="proj")
                        for r in range(4):
                            nc.tensor.matmul(
                                ps, wkf[r][:, 128 * h:128 * (h + 1)],
                                xtk[r][:, CH * n:CH * (n + 1)],
                                start=(r == 0), stop=(r == 3))
                        ts_(kp[h][:, CH * n:CH * (n + 1)], ps, bkp[:, h:h + 1],
                            None, OP.add)

                # head 0 first (gates the attention start), then v, then rest
                proj_q(0)
                proj_k(0)
                for i in range(NJ):
                    ps = ps_proj.tile([128, 512], f32, tag="proj", name="proj")
                    for r in range(4):
                        nc.tensor.matmul(
                            ps, xtv[r][:, 128 * i:128 * (i + 1)], wvf[r],
                            start=(r == 0), stop=(r == 3))
                    nc.vector.tensor_copy(
                        va[i][:, :, 0:64],
                        ps.rearrange("p (h d) -> p h d", d=64))
                    nc.gpsimd.memset(va[i][:, :, 64:65], 1.0)
                for h in range(1, 8):
                    proj_q(h)
                    proj_k(h)

                # gate: pre-gathered pair-packed weight tiles
                wgt = []
                for r in range(4):
                    w_t = wpool.tile([128, 512], bf16, tag=f"wg{r}", name=f"wg{r}", bufs=1)
                    nc.gpsimd.dma_start(
                        out=w_t,
                        in_=wq_d[128 * r:128 * (r + 1), :].rearrange(
                            "k (h blk) -> k h blk", blk=192)[:, :, 128:192])
                    wgt.append(w_t)
                for p in range(4):
                    for n in range(NN):
                        ps = ps_proj.tile([128, CH], f32, tag="proj", name="proj")
                        for r in range(4):
                            nc.tensor.matmul(
                                ps, wgt[r][:, 128 * p:128 * (p + 1)],
                                xtq[r][:, CH * n:CH * (n + 1)],
                                start=(r == 0), stop=(r == 3))
                        ts_(gt[p][:, CH * n:CH * (n + 1)], ps, bg[:, p:p + 1],
                            None, OP.add)

            # ---------- phase 2: attention, term-interleaved per head ------
            with tc.tile_pool(name="ps_att", bufs=2, space="PSUM") as ps_att, \
                 tc.tile_pool(name="ps_o", bufs=2, space="PSUM") as ps_o, \
                 tc.tile_pool(name="expp", bufs=4) as expp, \
                 tc.tile_pool(name="osp", bufs=2) as osp, \
                 tc.tile_pool(name="spp", bufs=1) as spp, \
                 tc.tile_pool(name="bcp", bufs=2) as bcp:

                sp_h = [spp.tile([2, S], f32, tag=f"sph{i}", name=f"sph{i}")
                        for i in range(2)]
                rp_h = [spp.tile([2, S], f32, tag=f"rph{i}", name=f"rph{i}")
                        for i in range(2)]
                rscr = spp.tile([2, S], f32, tag="rscr", name="rscr")

                for p in range(4):
                    for half in range(2):
                        h = 2 * p + half
                        # K=64 row-split scores: term1 on PE rows 0-63,
                        # term2 on rows 64-127, chunk-interleaved so the two
                        # streams overlap via row tiling.
                        o1 = ps_o.tile([65, S], f32, tag="o1", name="o1", bufs=1)
                        o2 = ps_o.tile([65, S], f32, tag="o2", name="o2", bufs=1)
                        for j in range(NJ):
                            s1 = ps_att.tile([128, S], f32, tag="s", name="s1")
                            s2 = ps_att.tile([128, S], f32, tag="s", name="s2")
                            for n in range(NN):
                                nc.tensor.matmul(
                                    s1[:, CH * n:CH * (n + 1)],
                                    kp[h][0:64, 128 * j:128 * (j + 1)],
                                    qp[h][0:64, CH * n:CH * (n + 1)],
                                    start=True, stop=True)
                                nc.tensor.matmul(
                                    s2[:, CH * n:CH * (n + 1)],
                                    kp[h][64:128, 128 * j:128 * (j + 1)],
                                    qp[h][64:128, CH * n:CH * (n + 1)],
                                    start=True, stop=True)
                            ex1 = expp.tile([128, S], bf16, tag="exp", name="ex1")
                            nc.scalar.activation(ex1, s1, AF.Exp, scale=INV)
                            ex2 = expp.tile([128, S], bf16, tag="exp", name="ex2")
                            nc.scalar.activation(ex2, s2, AF.Exp, scale=INV)
                            for n in range(NN):
                                nc.tensor.matmul(
                                    o1[:, CH * n:CH * (n + 1)],
                                    va[j][:, h, :],
                                    ex1[:, CH * n:CH * (n + 1)],
                                    start=(j == 0), stop=(j == NJ - 1))
                                nc.tensor.matmul(
                                    o2[:, CH * n:CH * (n + 1)],
                                    va[j][:, h, :],
                                    ex2[:, CH * n:CH * (n + 1)],
                                    start=(j == 0), stop=(j == NJ - 1))
                        os1 = osp.tile([65, S], f32, tag="os1", name="os1")
                        nc.vector.tensor_copy(os1, o1)
                        os2 = osp.tile([65, S], f32, tag="os2", name="os2")
                        nc.vector.tensor_copy(os2, o2)
                        sp = sp_h[half]
                        rp = rp_h[half]
                        nc.sync.dma_start(out=sp[0:1, :], in_=os1[64:65, :])
                        nc.sync.dma_start(out=sp[1:2, :], in_=os2[64:65, :])
                        # per-half reciprocal of the two exp-sums
                        nc.vector.reciprocal_approx_accurate(rp, sp, rscr)

                        ydst = ypair[p][64 * half:64 * half + 64, :]
                        if p < 3:
                            # broadcast 1/sum across 64 partitions on Pool
                            # (off the DVE/PE critical path)
                            r1 = bcp.tile([1, S], f32, tag="r1", name="r1")
                            nc.sync.dma_start(out=r1, in_=rp[0:1, :])
                            r2 = bcp.tile([1, S], f32, tag="r2", name="r2")
                            nc.sync.dma_start(out=r2, in_=rp[1:2, :])
                            bcs1 = bcp.tile([64, S], f32, tag="bcs1", name="bcs1")
                            nc.gpsimd.partition_broadcast(
                                bcs1, r1[0:1, :], channels=64)
                            bcs2 = bcp.tile([64, S], f32, tag="bcs2", name="bcs2")
                            nc.gpsimd.partition_broadcast(
                                bcs2, r2[0:1, :], channels=64)
                            nc.vector.tensor_mul(os1[0:64, :], os1[0:64, :], bcs1)
                            # os2 normalize on Pool (all-SBUF); -lam is
                            # applied in the DVE combine below
                            nc.gpsimd.tensor_mul(os2[0:64, :], os2[0:64, :], bcs2)
                        else:
                            # exposed tail: selector matmul broadcast in PSUM
                            bc = ps_att.tile([128, S], f32, tag="s", name="bcpe")
                            for n in range(NN):
                                nc.tensor.matmul(
                                    bc[:, CH * n:CH * (n + 1)], sel2,
                                    rp[:, CH * n:CH * (n + 1)],
                                    start=True, stop=True)
                            nc.vector.tensor_mul(
                                os1[0:64, :], os1[0:64, :], bc[0:64, :])
                            nc.vector.tensor_mul(
                                os2[0:64, :], os2[0:64, :], bc[64:128, :])
                        stt(ydst, os2[0:64, :], neglam64, os1[0:64, :],
                            OP.mult, OP.add, accum_out=sumcol[:, h:h + 1])
                        stt(os1[0:64, :], ydst, 1.0, ydst, OP.mult, OP.mult,
                            accum_out=sumcol[:, 8 + h:9 + h])

            # ---------- phase 3: gate tanh, stats, affine, output ----------
            with tc.tile_pool(name="tailp", bufs=1) as tailp, \
                 tc.tile_pool(name="oq", bufs=3) as oqp, \
                 tc.tile_pool(name="ps_tail", bufs=2, space="PSUM") as ps_tail:

                # tanh now: ACT is idle after the last exp, and the sqrt
                # table-load that follows overlaps the DVE stats chain
                th_t = [tailp.tile([128, S], f32, tag=f"th{p}", name=f"th{p}")
                        for p in range(4)]
                for p in range(4):
                    nc.scalar.activation(th_t[p], gt[p], AF.Tanh, scale=0.5)

                tot = tailp.tile([64, 2], f32, tag="tot", name="tot")
                nc.vector.tensor_reduce(
                    tot, sumcol.rearrange("p (t h) -> p t h", h=8),
                    axis=AX.X, op=OP.add)
                # bias-C (bv) corrections to the raw-Y stats
                csc = tailp.tile([64, 8], f32, tag="csc", name="csc")
                nc.vector.tensor_mul(csc, cc, sumcol[:, 0:8])
                cy64 = tailp.tile([64, 1], f32, tag="cy64", name="cy64")
                nc.vector.tensor_reduce(cy64, csc, axis=AX.X, op=OP.add)
                nc.vector.tensor_mul(csc, cc, cc)
                csq64 = tailp.tile([64, 1], f32, tag="csq64", name="csq64")
                nc.vector.tensor_reduce(csq64, csc, axis=AX.X, op=OP.add)
                csum64 = tailp.tile([64, 1], f32, tag="csum64", name="csum64")
                nc.vector.tensor_reduce(csum64, cc, axis=AX.X, op=OP.add)
                tot2 = tailp.tile([64, 2], f32, tag="tot2", name="tot2")
                stt(tot2[:, 0:1], csum64, float(S), tot[:, 0:1], OP.mult, OP.add)
                stt(tot2[:, 1:2], cy64, 2.0, tot[:, 1:2], OP.mult, OP.add)
                stt(tot2[:, 1:2], csq64, float(S), tot2[:, 1:2], OP.mult, OP.add)

                ms_ps = ps_tail.tile([64, 2], f32, tag="ms", name="ms")
                nc.tensor.matmul(ms_ps, ind2, tot2, start=True, stop=True)
                mean64 = tailp.tile([64, 1], f32, tag="mean64", name="mean64")
                ts_(mean64, ms_ps[:, 0:1], 1.0 / CNT, None, OP.mult)
                e264 = tailp.tile([64, 1], f32, tag="e264", name="e264")
                ts_(e264, ms_ps[:, 1:2], 1.0 / CNT, None, OP.mult)
                nm2 = tailp.tile([64, 1], f32, tag="nm2", name="nm2")
                ts_(nm2, mean64, mean64, -1.0, OP.mult, OP.mult)
                veps = tailp.tile([64, 1], f32, tag="veps", name="veps")
                stt(veps, nm2, EPS, e264, OP.add, OP.add)
                sd = tailp.tile([64, 1], f32, tag="sd", name="sd")
                nc.scalar.activation(sd, veps, AF.Sqrt)
                rsd = tailp.tile([64, 1], f32, tag="rsd", name="rsd")
                nc.vector.reciprocal(rsd, sd)
                # one Newton step for rsqrt accuracy (ACT sqrt is loose)
                rr = tailp.tile([64, 1], f32, tag="rr", name="rr")
                nc.vector.tensor_mul(rr, rsd, rsd)
                nc.vector.tensor_mul(rr, rr, veps)
                ts_(rr, rr, -0.5, 1.5, OP.mult, OP.add)
                rstd = tailp.tile([64, 1], f32, tag="rstd", name="rstd")
                nc.vector.tensor_mul(rstd, rsd, rr)

                a64 = tailp.tile([64, 1], f32, tag="a64", name="a64")
                ts_(a64, rstd, gamma_c, halfli, OP.mult, OP.mult)
                cm = tailp.tile([64, 8], f32, tag="cm", name="cm")
                ts_(cm, cc, mean64, None, OP.subtract)
                ball = tailp.tile([64, 8], f32, tag="ball", name="ball")
                ts_(ball, cm, a64, bb64, OP.mult, OP.add)

                for p in range(4):
                    for half in range(2):
                        h = 2 * p + half
                        rows = ypair[p][64 * half:64 * half + 64, :]
                        ts_(rows, rows, a64, ball[:, h:h + 1], OP.mult, OP.add)
                    # gate multiply on Pool (frees DVE for the affines)
                    nc.gpsimd.tensor_scalar_add(th_t[p], th_t[p], 1.0)
                    nc.gpsimd.tensor_mul(ypair[p], ypair[p], th_t[p])

                for c in range(NJ):
                    tp_o = ps_tail.tile([128, 512], f32, tag="tp_out", name="tp_out")
                    for p in range(4):
                        nc.tensor.transpose(
                            tp_o[:, 128 * p:128 * (p + 1)],
                            ypair[p][:, 128 * c:128 * (c + 1)], ident)
                    oq = oqp.tile([128, 512], f32, tag="oq", name="oq")
                    # PSUM->SBUF move on ACT (idle in the tail)
                    nc.scalar.copy(oq, tp_o)
                    nc.sync.dma_start(out=out_d[128 * c:128 * (c + 1), :], in_=oq)

    nc.finalize()
    return nc


_CACHE = {}


def _get_nc():
    if "nc" not in _CACHE:
        _CACHE["nc"] = build_nc(S_FULL)
    return _CACHE["nc"]


def run(inputs, trace=False, tmpdir=None):
    from concourse.bass_utils import run_bass_kernel_spmd
    nc = _get_nc()
    arrs = {k: np.asarray(v, dtype=np.float32) for k, v in inputs.items()}
    shared = {k: np.ascontiguousarray(arrs[k]) for k in
              ("Wq", "bq", "Wk", "bk", "Wv", "bv", "gamma", "beta",
               "lam", "lambda_init")}
    in_maps = []
    for i in range(B):
        m = dict(shared)
        m["query"] = np.ascontiguousarray(arrs["query"][i])
        m["key"] = np.ascontiguousarray(arrs["key"][i])
        m["values"] = np.ascontiguousarray(arrs["values"][i])
        in_maps.append(m)
    res = run_bass_kernel_spmd(nc, in_maps, core_ids=list(range(B)),
                               trace=trace, tmpdir=tmpdir)
    out = np.stack([res.results[i]["out"] for i in range(B)], axis=0)
    return out.astype(np.float32), res


def kernel(**inputs):
    out, _ = run(inputs)
    return out


# revision 17
# speedup vs baseline: 1.0538x; 1.0538x over previous
# Differential multi-head attention (dual softmax + GroupNorm + sigmoid gating)
# for Trainium2, batch-parallel across 8 NeuronCores (one batch row per core).
#
# Per-core math (batch b):
#   q = query @ Wq + bq -> per head: q1, q2, gate (each S x 64)
#   k = key   @ Wk + bk -> per head: k1, k2
#   v = values@ Wv + bv -> per head: v (S x 64)
#   attn = softmax(q1 k1^T / 8) - lam * softmax(q2 k2^T / 8)
#   out  = GroupNorm_{8 groups over d, reduced over (S, heads, d-in-group)}(attn @ v)
#   out  = out * (1 - lambda_init) * sigmoid(gate)
#
# Layout strategy: d-major ("transposed") attention: scores are computed as
# s^T (k on partitions, q free) so the attn@v contraction runs at K=128, and
# exp row-sums come free via a ones-column appended to v (M=65).
#
# Engine budget: ACT (scalar) is the bottleneck -- 128 exp tiles of
# [128,1024] at (N+352)/1.2GHz ~= 147us is the floor.  Everything else is
# kept off ACT: weight downcasts + x casts on Pool (gpsimd), projection
# epilogues + va/os copies + combines on DVE, gate tanh + output PSUM
# copies in the tail where ACT is idle.  Score matmuls are K=64 row-split
# (term1 rows 0-63, term2 rows 64-127, interleaved per chunk) so the PE
# streams both terms concurrently via row tiling.

import numpy as np

B, S_FULL, H, D = 8, 1024, 8, 64
DM = H * D  # 512


def build_nc(S=1024):
    import concourse.bacc as bacc
    import concourse.bass as bass
    import concourse.tile as tile
    from concourse import mybir
    from concourse.masks import make_identity

    f32 = mybir.dt.float32
    bf16 = mybir.dt.bfloat16
    AF = mybir.ActivationFunctionType
    OP = mybir.AluOpType
    AX = mybir.AxisListType

    NJ = S // 128          # k/seq 128-tiles
    CH = min(512, S)       # fp32-out matmul chunk
    NN = max(1, S // CH)
    CNT = float(S * H * (D // H))  # groupnorm reduction count per group
    EPS = 1e-3
    INV = 0.125            # 1/sqrt(64)

    nc = bacc.Bacc(target_bir_lowering=False)
    q_d = nc.dram_tensor("query", [S, DM], f32, kind="ExternalInput")
    k_d = nc.dram_tensor("key", [S, DM], f32, kind="ExternalInput")
    v_d = nc.dram_tensor("values", [S, DM], f32, kind="ExternalInput")
    wq_d = nc.dram_tensor("Wq", [DM, 3 * H * D], f32, kind="ExternalInput")
    bq_d = nc.dram_tensor("bq", [3 * H * D], f32, kind="ExternalInput")
    wk_d = nc.dram_tensor("Wk", [DM, 2 * H * D], f32, kind="ExternalInput")
    bk_d = nc.dram_tensor("bk", [2 * H * D], f32, kind="ExternalInput")
    wv_d = nc.dram_tensor("Wv", [DM, H * D], f32, kind="ExternalInput")
    bv_d = nc.dram_tensor("bv", [H * D], f32, kind="ExternalInput")
    gamma_d = nc.dram_tensor("gamma", [D], f32, kind="ExternalInput")
    beta_d = nc.dram_tensor("beta", [D], f32, kind="ExternalInput")
    lam_d = nc.dram_tensor("lam", [1], f32, kind="ExternalInput")
    li_d = nc.dram_tensor("lambda_init", [1], f32, kind="ExternalInput")
    out_d = nc.dram_tensor("out", [S, DM], f32, kind="ExternalOutput")

    ts_ = nc.vector.tensor_scalar
    stt = nc.vector.scalar_tensor_tensor

    with tile.TileContext(nc) as tc:
        with tc.tile_pool(name="consts", bufs=1) as consts, \
             tc.tile_pool(name="persist", bufs=1) as persist:

            # ---------- constants ----------
            # dummy exp to pull the ACT exp/tanh table load to t=0
            dmy = consts.tile([1, 8], f32, tag="dmy", name="dmy")
            nc.gpsimd.memset(dmy, 0.0)
            dmyo = consts.tile([1, 8], f32, tag="dmyo", name="dmyo")
            nc.scalar.activation(dmyo, dmy, AF.Exp)

            ident = consts.tile([128, 128], f32, tag="ident", name="ident")
            make_identity(nc, ident)
            ident_b = consts.tile([128, 128], bf16, tag="ident_b", name="ident_b")
            make_identity(nc, ident_b)

            # block-diagonal group matrix: IND2[d', d] = 1 iff d'//8 == d//8
            ind2 = consts.tile([64, 64], f32, tag="ind2", name="ind2")
            nc.gpsimd.memset(ind2, 1.0)
            nc.gpsimd.affine_select(
                out=ind2, in_=ind2, compare_op=OP.is_ge, fill=0.0,
                base=0, pattern=[[-8, 8], [0, 8]], channel_multiplier=1)
            nc.gpsimd.affine_select(
                out=ind2, in_=ind2, compare_op=OP.is_ge, fill=0.0,
                base=7, pattern=[[8, 8], [0, 8]], channel_multiplier=-1)

            # selector for the r-row broadcast matmul (used by the last pair)
            # sel2[p, x] = 1 iff (x - 64p) in [0, 64): row 0 -> parts 0-63,
            # row 1 -> parts 64-127
            sel2 = consts.tile([2, 128], f32, tag="sel2", name="sel2")
            nc.gpsimd.memset(sel2, 1.0)
            nc.gpsimd.affine_select(
                out=sel2, in_=sel2, compare_op=OP.is_ge, fill=0.0,
                base=0, pattern=[[1, 128]], channel_multiplier=-64)
            nc.gpsimd.affine_select(
                out=sel2, in_=sel2, compare_op=OP.is_ge, fill=0.0,
                base=63, pattern=[[-1, 128]], channel_multiplier=64)

            # scalar columns
            lam64 = consts.tile([64, 1], f32, tag="lam64", name="lam64")
            nc.gpsimd.dma_start(out=lam64, in_=lam_d[:].to_broadcast([64, 1]))
            li64 = consts.tile([64, 1], f32, tag="li64", name="li64")
            nc.gpsimd.dma_start(out=li64, in_=li_d[:].to_broadcast([64, 1]))
            neglam64 = consts.tile([64, 1], f32, tag="neglam64", name="neglam64")
            ts_(neglam64, lam64, -1.0, None, OP.mult)
            onelam64 = consts.tile([64, 1], f32, tag="onelam64", name="onelam64")
            ts_(onelam64, lam64, -1.0, 1.0, OP.mult, OP.add)   # 1 - lam
            halfli = consts.tile([64, 1], f32, tag="halfli", name="halfli")
            ts_(halfli, li64, -0.5, 0.5, OP.mult, OP.add)      # 0.5*(1-li)

            gamma_c = consts.tile([64, 1], f32, tag="gamma_c", name="gamma_c")
            nc.sync.dma_start(out=gamma_c, in_=gamma_d[:])
            beta_c = consts.tile([64, 1], f32, tag="beta_c", name="beta_c")
            nc.sync.dma_start(out=beta_c, in_=beta_d[:])
            bb64 = consts.tile([64, 1], f32, tag="bb64", name="bb64")
            ts_(bb64, beta_c, halfli, None, OP.mult)           # beta*0.5*(1-li)

            # v-bias columns per head and C = bv*(1-lam) fold
            bvc = consts.tile([64, 8], f32, tag="bvc", name="bvc")
            nc.sync.dma_start(
                out=bvc, in_=bv_d[:].rearrange("(h d) -> d h", d=64))
            cc = consts.tile([64, 8], f32, tag="cc", name="cc")
            ts_(cc, bvc, onelam64, None, OP.mult)

            # bias columns: per-head stacked [q1|q2] / [k1|k2] are contiguous
            # 128-element runs of bq/bk; gate needs a gathered layout.
            bqp = consts.tile([128, 8], f32, tag="bqp", name="bqp")
            nc.sync.dma_start(
                out=bqp,
                in_=bq_d[:].rearrange("(h blk) -> blk h", blk=192)[0:128, :])
            bkp = consts.tile([128, 8], f32, tag="bkp", name="bkp")
            nc.sync.dma_start(
                out=bkp,
                in_=bk_d[:].rearrange("(h blk) -> blk h", blk=128))
            bg = consts.tile([128, 4], f32, tag="bg", name="bg")
            bqv = bq_d[:].rearrange("(h blk) -> h blk", blk=192)
            for p in range(4):
                nc.sync.dma_start(out=bg[:, p:p + 1],
                                  in_=bqv[2 * p:2 * p + 2, 128:192])

            # persistent projection outputs (bf16, d-major)
            # qp/kp[h]: rows 0-63 = q1/k1 of head h, rows 64-127 = q2/k2
            qp = [persist.tile([128, S], bf16, tag=f"qp{h}", name=f"qp{h}") for h in range(8)]
            kp = [persist.tile([128, S], bf16, tag=f"kp{h}", name=f"kp{h}") for h in range(8)]
            # gate stays head-pair packed: gt[p] rows 0-63 = head 2p, 64-127 = 2p+1
            gt = [persist.tile([128, S], bf16, tag=f"gt{p}", name=f"gt{p}") for p in range(4)]
            va = [persist.tile([128, 8, 65], bf16, tag=f"va{i}", name=f"va{i}") for i in range(NJ)]
            ypair = [persist.tile([128, S], f32, tag=f"yp{p}", name=f"yp{p}") for p in range(4)]
            sumcol = persist.tile([64, 16], f32, tag="sumcol", name="sumcol")

            # ---------- phase 1: load + cast + transpose + project ----------
            # Interleaved q/k group loading so head-0's projections (and the
            # first exp) fire ~10us in.  Head-0 slices of Wq/Wk are DMA'd
            # and ACT-cast first; bulk weight columns cast on DVE behind.
            GRP = min(4, NJ)
            NG = NJ // GRP
            with tc.tile_pool(name="xin", bufs=3) as xin_pool, \
                 tc.tile_pool(name="xtp", bufs=1) as xtp, \
                 tc.tile_pool(name="wload", bufs=2) as wpool, \
                 tc.tile_pool(name="ps_in", bufs=1, space="PSUM") as ps_in, \
                 tc.tile_pool(name="ps_proj", bufs=4, space="PSUM") as ps_proj:

                wqf = [wpool.tile([128, 3 * H * D], bf16, tag=f"wqf{r}", name=f"wqf{r}", bufs=1) for r in range(4)]
                wkf = [wpool.tile([128, 2 * H * D], bf16, tag=f"wkf{r}", name=f"wkf{r}", bufs=1) for r in range(4)]
                wvf = [wpool.tile([128, H * D], bf16, tag=f"wvf{r}", name=f"wvf{r}", bufs=1) for r in range(4)]

                for r in range(4):
                    ws0 = wpool.tile([128, 192], f32, tag="wsq0", name="wsq0")
                    nc.sync.dma_start(out=ws0, in_=wq_d[128 * r:128 * (r + 1), 0:192])
                    nc.scalar.copy(wqf[r][:, 0:192], ws0)
                for r in range(4):
                    ws0 = wpool.tile([128, 128], f32, tag="wsk0", name="wsk0")
                    nc.sync.dma_start(out=ws0, in_=wk_d[128 * r:128 * (r + 1), 0:128])
                    nc.scalar.copy(wkf[r][:, 0:128], ws0)

                xtq = [xtp.tile([128, S], bf16, tag=f"xtq{c}", name=f"xtq{c}") for c in range(4)]
                xtk = [xtp.tile([128, S], bf16, tag=f"xtk{c}", name=f"xtk{c}") for c in range(4)]
                xtv = [xtp.tile([128, S], bf16, tag=f"xtv{c}", name=f"xtv{c}") for c in range(4)]

                def grp_block(x_dram, xt, g, nm):
                    tp_cur = [ps_in.tile([128, 128 * GRP], bf16,
                                         tag=f"tp{c}", name=f"tp{nm}{c}")
                              for c in range(4)]
                    for ii in range(GRP):
                        i = GRP * g + ii
                        xs = xin_pool.tile([128, DM], f32, tag="xs", name="xs")
                        nc.sync.dma_start(out=xs, in_=x_dram[128 * i:128 * (i + 1), :])
                        xq = xin_pool.tile([128, DM], bf16, tag="xin", name="xin")
                        nc.vector.tensor_copy(xq, xs)
                        for c in range(4):
                            nc.tensor.transpose(
                                tp_cur[c][:, 128 * ii:128 * (ii + 1)],
                                xq[:, 128 * c:128 * (c + 1)], ident_b)
                    base = 128 * GRP * g
                    for c in range(4):
                        nc.vector.tensor_copy(
                            xt[c][:, base:base + 128 * GRP], tp_cur[c])

                def proj_q(h, n):
                    ps = ps_proj.tile([128, CH], f32, tag="proj", name="proj")
                    for r in range(4):
                        nc.tensor.matmul(
                            ps, wqf[r][:, 192 * h:192 * h + 128],
                            xtq[r][:, CH * n:CH * (n + 1)],
                            start=(r == 0), stop=(r == 3))
                    ts_(qp[h][:, CH * n:CH * (n + 1)], ps, bqp[:, h:h + 1],
                        None, OP.add)

                def proj_k(h, n):
                    ps = ps_proj.tile([128, CH], f32, tag="proj", name="proj")
                    for r in range(4):
                        nc.tensor.matmul(
                            ps, wkf[r][:, 128 * h:128 * (h + 1)],
                            xtk[r][:, CH * n:CH * (n + 1)],
                            start=(r == 0), stop=(r == 3))
                    ts_(kp[h][:, CH * n:CH * (n + 1)], ps, bkp[:, h:h + 1],
                        None, OP.add)

                def proj_v(i):
                    ps = ps_proj.tile([128, 512], f32, tag="proj", name="proj")
                    for r in range(4):
                        nc.tensor.matmul(
                            ps, xtv[r][:, 128 * i:128 * (i + 1)], wvf[r],
                            start=(r == 0), stop=(r == 3))
                    nc.vector.tensor_copy(
                        va[i][:, :, 0:64],
                        ps.rearrange("p (h d) -> p h d", d=64))
                    nc.gpsimd.memset(va[i][:, :, 64:65], 1.0)

                # group 0 of q and k -> head-0 chunk-0 projections
                grp_block(q_d, xtq, 0, "q")
                grp_block(k_d, xtk, 0, "k")
                proj_q(0, 0)
                proj_k(0, 0)
                # wv loads now (DVE casts), v data behind q/k group 1
                wsv_s = []
                for r in range(4):
                    wsv = wpool.tile([128, H * D], f32, tag="wsv", name="wsv")
                    nc.sync.dma_start(out=wsv, in_=wv_d[128 * r:128 * (r + 1), :])
                    wsv_s.append(wsv)
                for g in range(1, NG):
                    grp_block(q_d, xtq, g, "q")
                    grp_block(k_d, xtk, g, "k")
                for r in range(4):
                    nc.vector.tensor_copy(wvf[r], wsv_s[r])
                for n in range(1, NN):
                    proj_q(0, n)
                    proj_k(0, n)
                for g in range(NG):
                    grp_block(v_d, xtv, g, "v")
                    for i in range(GRP * g, GRP * (g + 1)):
                        proj_v(i)
                # bulk weight columns: DMA + DVE casts
                for r in range(4):
                    wsq = wpool.tile([128, 3 * H * D - 192], f32, tag="wsqr", name="wsqr")
                    nc.sync.dma_start(out=wsq, in_=wq_d[128 * r:128 * (r + 1), 192:])
                    nc.vector.tensor_copy(wqf[r][:, 192:], wsq)
                    wsk = wpool.tile([128, 2 * H * D - 128], f32, tag="wskr", name="wskr")
                    nc.sync.dma_start(out=wsk, in_=wk_d[128 * r:128 * (r + 1), 128:])
                    nc.vector.tensor_copy(wkf[r][:, 128:], wsk)
                for h in range(1, 8):
                    for n in range(NN):
                        proj_q(h, n)
                        proj_k(h, n)

                # gate: pre-gathered pair-packed weight tiles
                wgt = []
                for r in range(4):
                    w_t = wpool.tile([128, 512], bf16, tag=f"wg{r}", name=f"wg{r}", bufs=1)
                    nc.gpsimd.dma_start(
                        out=w_t,
                        in_=wq_d[128 * r:128 * (r + 1), :].rearrange(
                            "k (h blk) -> k h blk", blk=192)[:, :, 128:192])
                    wgt.append(w_t)
                for p in range(4):
                    for n in range(NN):
                        ps = ps_proj.tile([128, CH], f32, tag="proj", name="proj")
                        for r in range(4):
                            nc.tensor.matmul(
                                ps, wgt[r][:, 128 * p:128 * (p + 1)],
                                xtq[r][:, CH * n:CH * (n + 1)],
                                start=(r == 0), stop=(r == 3))
                        ts_(gt[p][:, CH * n:CH * (n + 1)], ps, bg[:, p:p + 1],
                            None, OP.add)

            # ---------- phase 2: attention, term-interleaved per head ------
            with tc.tile_pool(name="ps_att", bufs=2, space="PSUM") as ps_att, \
                 tc.tile_pool(name="ps_o", bufs=2, space="PSUM") as ps_o, \
                 tc.tile_pool(name="expp", bufs=4) as expp, \
                 tc.tile_pool(name="osp", bufs=2) as osp, \
                 tc.tile_pool(name="spp", bufs=1) as spp, \
                 tc.tile_pool(name="bcp", bufs=2) as bcp:

                sp_h = [spp.tile([2, S], f32, tag=f"sph{i}", name=f"sph{i}")
                        for i in range(2)]
                rp_h = [spp.tile([2, S], f32, tag=f"rph{i}", name=f"rph{i}")
                        for i in range(2)]
                rscr = spp.tile([2, S], f32, tag="rscr", name="rscr")

                for p in range(4):
                    for half in range(2):
                        h = 2 * p + half
                        # K=64 row-split scores: term1 on PE rows 0-63,
                        # term2 on rows 64-127, chunk-interleaved so the two
                        # streams overlap via row tiling.
                        o1 = ps_o.tile([65, S], f32, tag="o1", name="o1", bufs=1)
                        o2 = ps_o.tile([65, S], f32, tag="o2", name="o2", bufs=1)
                        for j in range(NJ):
                            s1 = ps_att.tile([128, S], f32, tag="s", name="s1")
                            s2 = ps_att.tile([128, S], f32, tag="s", name="s2")
                            for n in range(NN):
                                nc.tensor.matmul(
                                    s1[:, CH * n:CH * (n + 1)],
                                    kp[h][0:64, 128 * j:128 * (j + 1)],
                                    qp[h][0:64, CH * n:CH * (n + 1)],
                                    start=True, stop=True)
                                nc.tensor.matmul(
                                    s2[:, CH * n:CH * (n + 1)],
                                    kp[h][64:128, 128 * j:128 * (j + 1)],
                                    qp[h][64:128, CH * n:CH * (n + 1)],
                                    start=True, stop=True)
                            ex1 = expp.tile([128, S], bf16, tag="exp", name="ex1")
                            nc.scalar.activation(ex1, s1, AF.Exp, scale=INV)
                            ex2 = expp.tile([128, S], bf16, tag="exp", name="ex2")
                            nc.scalar.activation(ex2, s2, AF.Exp, scale=INV)
                            for n in range(NN):
                                nc.tensor.matmul(
                                    o1[:, CH * n:CH * (n + 1)],
                                    va[j][:, h, :],
                                    ex1[:, CH * n:CH * (n + 1)],
                                    start=(j == 0), stop=(j == NJ - 1))
                                nc.tensor.matmul(
                                    o2[:, CH * n:CH * (n + 1)],
                                    va[j][:, h, :],
                                    ex2[:, CH * n:CH * (n + 1)],
                                    start=(j == 0), stop=(j == NJ - 1))
                        os1 = osp.tile([65, S], f32, tag="os1", name="os1")
                        nc.vector.tensor_copy(os1, o1)
                        os2 = osp.tile([65, S], f32, tag="os2", name="os2")
                        nc.vector.tensor_copy(os2, o2)
                        sp = sp_h[half]
                        rp = rp_h[half]
                        nc.sync.dma_start(out=sp[0:1, :], in_=os1[64:65, :])
                        nc.sync.dma_start(out=sp[1:2, :], in_=os2[64:65, :])
                        # per-half reciprocal of the two exp-sums
                        nc.vector.reciprocal_approx_accurate(rp, sp, rscr)

                        ydst = ypair[p][64 * half:64 * half + 64, :]
                        if p < 3:
                            # broadcast 1/sum across 64 partitions on Pool
                            # (off the DVE/PE critical path)
                            r1 = bcp.tile([1, S], f32, tag="r1", name="r1")
                            nc.sync.dma_start(out=r1, in_=rp[0:1, :])
                            r2 = bcp.tile([1, S], f32, tag="r2", name="r2")
                            nc.sync.dma_start(out=r2, in_=rp[1:2, :])
                            bcs1 = bcp.tile([64, S], f32, tag="bcs1", name="bcs1")
                            nc.gpsimd.partition_broadcast(
                                bcs1, r1[0:1, :], channels=64)
                            bcs2 = bcp.tile([64, S], f32, tag="bcs2", name="bcs2")
                            nc.gpsimd.partition_broadcast(
                                bcs2, r2[0:1, :], channels=64)
                            nc.vector.tensor_mul(os1[0:64, :], os1[0:64, :], bcs1)
                            nc.vector.tensor_mul(os2[0:64, :], os2[0:64, :], bcs2)
                        else:
                            # exposed tail: selector matmul broadcast in PSUM
                            bc = ps_att.tile([128, S], f32, tag="s", name="bcpe")
                            for n in range(NN):
                                nc.tensor.matmul(
                                    bc[:, CH * n:CH * (n + 1)], sel2,
                                    rp[:, CH * n:CH * (n + 1)],
                                    start=True, stop=True)
                            nc.vector.tensor_mul(
                                os1[0:64, :], os1[0:64, :], bc[0:64, :])
                            nc.vector.tensor_mul(
                                os2[0:64, :], os2[0:64, :], bc[64:128, :])
                        stt(ydst, os2[0:64, :], neglam64, os1[0:64, :],
                            OP.mult, OP.add, accum_out=sumcol[:, h:h + 1])
                        stt(os1[0:64, :], ydst, 1.0, ydst, OP.mult, OP.mult,
                            accum_out=sumcol[:, 8 + h:9 + h])

            # ---------- phase 3: gate tanh, stats, affine, output ----------
            with tc.tile_pool(name="tailp", bufs=1) as tailp, \
                 tc.tile_pool(name="oq", bufs=3) as oqp, \
                 tc.tile_pool(name="ps_tail", bufs=2, space="PSUM") as ps_tail:

                # tanh now: ACT is idle after the last exp, and the sqrt
                # table-load that follows overlaps the DVE stats chain
                th_t = [tailp.tile([128, S], f32, tag=f"th{p}", name=f"th{p}")
                        for p in range(4)]
                for p in range(4):
                    nc.scalar.activation(th_t[p], gt[p], AF.Tanh, scale=0.5)

                tot = tailp.tile([64, 2], f32, tag="tot", name="tot")
                nc.vector.tensor_reduce(
                    tot, sumcol.rearrange("p (t h) -> p t h", h=8),
                    axis=AX.X, op=OP.add)
                # bias-C (bv) corrections to the raw-Y stats
                csc = tailp.tile([64, 8], f32, tag="csc", name="csc")
                nc.vector.tensor_mul(csc, cc, sumcol[:, 0:8])
                cy64 = tailp.tile([64, 1], f32, tag="cy64", name="cy64")
                nc.vector.tensor_reduce(cy64, csc, axis=AX.X, op=OP.add)
                nc.vector.tensor_mul(csc, cc, cc)
                csq64 = tailp.tile([64, 1], f32, tag="csq64", name="csq64")
                nc.vector.tensor_reduce(csq64, csc, axis=AX.X, op=OP.add)
                csum64 = tailp.tile([64, 1], f32, tag="csum64", name="csum64")
                nc.vector.tensor_reduce(csum64, cc, axis=AX.X, op=OP.add)
                tot2 = tailp.tile([64, 2], f32, tag="tot2", name="tot2")
                stt(tot2[:, 0:1], csum64, float(S), tot[:, 0:1], OP.mult, OP.add)
                stt(tot2[:, 1:2], cy64, 2.0, tot[:, 1:2], OP.mult, OP.add)
                stt(tot2[:, 1:2], csq64, float(S), tot2[:, 1:2], OP.mult, OP.add)

                ms_ps = ps_tail.tile([64, 2], f32, tag="ms", name="ms")
                nc.tensor.matmul(ms_ps, ind2, tot2, start=True, stop=True)
                mean64 = tailp.tile([64, 1], f32, tag="mean64", name="mean64")
                ts_(mean64, ms_ps[:, 0:1], 1.0 / CNT, None, OP.mult)
                e264 = tailp.tile([64, 1], f32, tag="e264", name="e264")
                ts_(e264, ms_ps[:, 1:2], 1.0 / CNT, None, OP.mult)
                nm2 = tailp.tile([64, 1], f32, tag="nm2", name="nm2")
                ts_(nm2, mean64, mean64, -1.0, OP.mult, OP.mult)
                veps = tailp.tile([64, 1], f32, tag="veps", name="veps")
                stt(veps, nm2, EPS, e264, OP.add, OP.add)
                sd = tailp.tile([64, 1], f32, tag="sd", name="sd")
                nc.scalar.activation(sd, veps, AF.Sqrt)
                rsd = tailp.tile([64, 1], f32, tag="rsd", name="rsd")
                nc.vector.reciprocal(rsd, sd)
                # one Newton step for rsqrt accuracy (ACT sqrt is loose)
                rr = tailp.tile([64, 1], f32, tag="rr", name="rr")
                nc.vector.tensor_mul(rr, rsd, rsd)
                nc.vector.tensor_mul(rr, rr, veps)
                ts_(rr, rr, -0.5, 1.5, OP.mult, OP.add)
                rstd = tailp.tile([64, 1], f32, tag="rstd", name="rstd")
                nc.vector.tensor_mul(rstd, rsd, rr)

                a64 = tailp.tile([64, 1], f32, tag="a64", name="a64")
                ts_(a64, rstd, gamma_c, halfli, OP.mult, OP.mult)
                cm = tailp.tile([64, 8], f32, tag="cm", name="cm")
                ts_(cm, cc, mean64, None, OP.subtract)
                ball = tailp.tile([64, 8], f32, tag="ball", name="ball")
                ts_(ball, cm, a64, bb64, OP.mult, OP.add)

                for p in range(4):
                    for half in range(2):
                        h = 2 * p + half
                        rows = ypair[p][64 * half:64 * half + 64, :]
                        ts_(rows, rows, a64, ball[:, h:h + 1], OP.mult, OP.add)
                    stt(ypair[p], th_t[p], 1.0, ypair[p], OP.add, OP.mult)

                for c in range(NJ):
                    tp_o = ps_tail.tile([128, 512], f32, tag="tp_out", name="tp_out")
                    for p in range(4):
                        nc.tensor.transpose(
                            tp_o[:, 128 * p:128 * (p + 1)],
                            ypair[p][:, 128 * c:128 * (c + 1)], ident)
                    oq = oqp.tile([128, 512], f32, tag="oq", name="oq")
                    # PSUM->SBUF move on ACT (idle in the tail)
                    nc.scalar.copy(oq, tp_o)
                    nc.sync.dma_start(out=out_d[128 * c:128 * (c + 1), :], in_=oq)

    nc.finalize()
    return nc


_CACHE = {}


def _get_nc():
    if "nc" not in _CACHE:
        _CACHE["nc"] = build_nc(S_FULL)
    return _CACHE["nc"]


def run(inputs, trace=False, tmpdir=None):
    from concourse.bass_utils import run_bass_kernel_spmd
    nc = _get_nc()
    arrs = {k: np.asarray(v, dtype=np.float32) for k, v in inputs.items()}
    shared = {k: np.ascontiguousarray(arrs[k]) for k in
              ("Wq", "bq", "Wk", "bk", "Wv", "bv", "gamma", "beta",
               "lam", "lambda_init")}
    in_maps = []
    for i in range(B):
        m = dict(shared)
        m["query"] = np.ascontiguousarray(arrs["query"][i])
        m["key"] = np.ascontiguousarray(arrs["key"][i])
        m["values"] = np.ascontiguousarray(arrs["values"][i])
        in_maps.append(m)
    res = run_bass_kernel_spmd(nc, in_maps, core_ids=list(range(B)),
                               trace=trace, tmpdir=tmpdir)
    out = np.stack([res.results[i]["out"] for i in range(B)], axis=0)
    return out.astype(np.float32), res


def kernel(**inputs):
    out, _ = run(inputs)
    return out


# revision 21
# speedup vs baseline: 1.5513x; 1.4720x over previous
# Differential multi-head attention (dual softmax + GroupNorm + sigmoid gating)
# for Trainium2, batch-parallel across 8 NeuronCores (one batch row per core).
#
# Per-core math (batch b):
#   q = query @ Wq + bq -> per head: q1, q2, gate (each S x 64)
#   k = key   @ Wk + bk -> per head: k1, k2
#   v = values@ Wv + bv -> per head: v (S x 64)
#   attn = softmax(q1 k1^T / 8) - lam * softmax(q2 k2^T / 8)
#   out  = GroupNorm_{8 groups over d, reduced over (S, heads, d-in-group)}(attn @ v)
#   out  = out * (1 - lambda_init) * sigmoid(gate)
#
# Layout strategy: d-major ("transposed") attention: scores are computed as
# s^T (k on partitions, q free) so the attn@v contraction runs at K=128, and
# exp row-sums come free via a ones-column appended to v (M=65).
#
# Engine budget: ACT (scalar) is the bottleneck -- 128 exp tiles of
# [128,1024] at (N+352)/1.2GHz ~= 147us is the floor.  Everything else is
# kept off ACT: weight downcasts + x casts on Pool (gpsimd), projection
# epilogues + va/os copies + combines on DVE, gate tanh + output PSUM
# copies in the tail where ACT is idle.  Score matmuls are K=64 row-split
# (term1 rows 0-63, term2 rows 64-127, interleaved per chunk) so the PE
# streams both terms concurrently via row tiling.

import numpy as np

B, S_FULL, H, D = 8, 1024, 8, 64
DM = H * D  # 512


def build_nc(S=1024):
    import concourse.bacc as bacc
    import concourse.bass as bass
    import concourse.tile as tile
    from concourse import mybir
    from concourse.masks import make_identity

    f32 = mybir.dt.float32
    bf16 = mybir.dt.bfloat16
    AF = mybir.ActivationFunctionType
    OP = mybir.AluOpType
    AX = mybir.AxisListType

    NJ = S // 128          # k/seq 128-tiles
    CH = min(512, S)       # fp32-out matmul chunk
    NN = max(1, S // CH)
    CNT = float(S * H * (D // H))  # groupnorm reduction count per group
    EPS = 1e-3
    INV = 0.125            # 1/sqrt(64)

    nc = bacc.Bacc(target_bir_lowering=False)
    q_d = nc.dram_tensor("query", [S, DM], f32, kind="ExternalInput")
    k_d = nc.dram_tensor("key", [S, DM], f32, kind="ExternalInput")
    v_d = nc.dram_tensor("values", [S, DM], f32, kind="ExternalInput")
    wq_d = nc.dram_tensor("Wq", [DM, 3 * H * D], f32, kind="ExternalInput")
    bq_d = nc.dram_tensor("bq", [3 * H * D], f32, kind="ExternalInput")
    wk_d = nc.dram_tensor("Wk", [DM, 2 * H * D], f32, kind="ExternalInput")
    bk_d = nc.dram_tensor("bk", [2 * H * D], f32, kind="ExternalInput")
    wv_d = nc.dram_tensor("Wv", [DM, H * D], f32, kind="ExternalInput")
    bv_d = nc.dram_tensor("bv", [H * D], f32, kind="ExternalInput")
    gamma_d = nc.dram_tensor("gamma", [D], f32, kind="ExternalInput")
    beta_d = nc.dram_tensor("beta", [D], f32, kind="ExternalInput")
    lam_d = nc.dram_tensor("lam", [1], f32, kind="ExternalInput")
    li_d = nc.dram_tensor("lambda_init", [1], f32, kind="ExternalInput")
    out_d = nc.dram_tensor("out", [S, DM], f32, kind="ExternalOutput")

    ts_ = nc.vector.tensor_scalar
    stt = nc.vector.scalar_tensor_tensor

    with tile.TileContext(nc) as tc:
        with tc.tile_pool(name="consts", bufs=1) as consts, \
             tc.tile_pool(name="persist", bufs=1) as persist:

            # ---------- constants ----------
            # dummy exp to pull the ACT exp/tanh table load to t=0
            dmy = consts.tile([1, 8], f32, tag="dmy", name="dmy")
            nc.gpsimd.memset(dmy, 0.0)
            dmyo = consts.tile([1, 8], f32, tag="dmyo", name="dmyo")
            nc.scalar.activation(dmyo, dmy, AF.Exp)

            ident = consts.tile([128, 128], f32, tag="ident", name="ident")
            make_identity(nc, ident)
            ident_b = consts.tile([128, 128], bf16, tag="ident_b", name="ident_b")
            make_identity(nc, ident_b)

            # block-diagonal group matrix: IND2[d', d] = 1 iff d'//8 == d//8
            ind2 = consts.tile([64, 64], f32, tag="ind2", name="ind2")
            nc.gpsimd.memset(ind2, 1.0)
            nc.gpsimd.affine_select(
                out=ind2, in_=ind2, compare_op=OP.is_ge, fill=0.0,
                base=0, pattern=[[-8, 8], [0, 8]], channel_multiplier=1)
            nc.gpsimd.affine_select(
                out=ind2, in_=ind2, compare_op=OP.is_ge, fill=0.0,
                base=7, pattern=[[8, 8], [0, 8]], channel_multiplier=-1)

            # selector for the r-row broadcast matmul (used by the last pair)
            # sel2[p, x] = 1 iff (x - 64p) in [0, 64): row 0 -> parts 0-63,
            # row 1 -> parts 64-127
            sel2 = consts.tile([2, 128], f32, tag="sel2", name="sel2")
            nc.gpsimd.memset(sel2, 1.0)
            nc.gpsimd.affine_select(
                out=sel2, in_=sel2, compare_op=OP.is_ge, fill=0.0,
                base=0, pattern=[[1, 128]], channel_multiplier=-64)
            nc.gpsimd.affine_select(
                out=sel2, in_=sel2, compare_op=OP.is_ge, fill=0.0,
                base=63, pattern=[[-1, 128]], channel_multiplier=64)

            # scalar columns
            lam64 = consts.tile([64, 1], f32, tag="lam64", name="lam64")
            nc.gpsimd.dma_start(out=lam64, in_=lam_d[:].to_broadcast([64, 1]))
            li64 = consts.tile([64, 1], f32, tag="li64", name="li64")
            nc.gpsimd.dma_start(out=li64, in_=li_d[:].to_broadcast([64, 1]))
            neglam64 = consts.tile([64, 1], f32, tag="neglam64", name="neglam64")
            ts_(neglam64, lam64, -1.0, None, OP.mult)
            onelam64 = consts.tile([64, 1], f32, tag="onelam64", name="onelam64")
            ts_(onelam64, lam64, -1.0, 1.0, OP.mult, OP.add)   # 1 - lam
            halfli = consts.tile([64, 1], f32, tag="halfli", name="halfli")
            ts_(halfli, li64, -0.5, 0.5, OP.mult, OP.add)      # 0.5*(1-li)

            gamma_c = consts.tile([64, 1], f32, tag="gamma_c", name="gamma_c")
            nc.sync.dma_start(out=gamma_c, in_=gamma_d[:])
            beta_c = consts.tile([64, 1], f32, tag="beta_c", name="beta_c")
            nc.sync.dma_start(out=beta_c, in_=beta_d[:])
            bb64 = consts.tile([64, 1], f32, tag="bb64", name="bb64")
            ts_(bb64, beta_c, halfli, None, OP.mult)           # beta*0.5*(1-li)

            # v-bias columns per head and C = bv*(1-lam) fold
            bvc = consts.tile([64, 8], f32, tag="bvc", name="bvc")
            nc.sync.dma_start(
                out=bvc, in_=bv_d[:].rearrange("(h d) -> d h", d=64))
            cc = consts.tile([64, 8], f32, tag="cc", name="cc")
            ts_(cc, bvc, onelam64, None, OP.mult)

            # bias columns: per-head stacked [q1|q2] / [k1|k2] are contiguous
            # 128-element runs of bq/bk; gate needs a gathered layout.
            bqp = consts.tile([128, 8], f32, tag="bqp", name="bqp")
            nc.sync.dma_start(
                out=bqp,
                in_=bq_d[:].rearrange("(h blk) -> blk h", blk=192)[0:128, :])
            bkp = consts.tile([128, 8], f32, tag="bkp", name="bkp")
            nc.sync.dma_start(
                out=bkp,
                in_=bk_d[:].rearrange("(h blk) -> blk h", blk=128))
            bg = consts.tile([128, 4], f32, tag="bg", name="bg")
            bqv = bq_d[:].rearrange("(h blk) -> h blk", blk=192)
            for p in range(4):
                nc.sync.dma_start(out=bg[:, p:p + 1],
                                  in_=bqv[2 * p:2 * p + 2, 128:192])

            # persistent projection outputs (bf16, d-major)
            # qp/kp[h]: rows 0-63 = q1/k1 of head h, rows 64-127 = q2/k2
            qp = [persist.tile([128, S], bf16, tag=f"qp{h}", name=f"qp{h}") for h in range(8)]
            kp = [persist.tile([128, S], bf16, tag=f"kp{h}", name=f"kp{h}") for h in range(8)]
            # gate stays head-pair packed: gt[p] rows 0-63 = head 2p, 64-127 = 2p+1
            gt = [persist.tile([128, S], bf16, tag=f"gt{p}", name=f"gt{p}") for p in range(4)]
            va = [persist.tile([128, 8, 65], bf16, tag=f"va{i}", name=f"va{i}") for i in range(NJ)]
            ypair = [persist.tile([128, S], f32, tag=f"yp{p}", name=f"yp{p}") for p in range(4)]
            sumcol = persist.tile([64, 16], f32, tag="sumcol", name="sumcol")

            # ---------- unified PSUM pool: exactly 16KB of tag rings ----
            # tp   [128,512] bf16 x2 = 2KB   (input transposes)
            # proj [128,512] f32  x1 = 2KB   (projection accumulator)
            # s    [128,1024] f32 x2 = 8KB   (scores / bc / tail transposes)
            # o    [65,1024] f32  x1 = 4KB   (attn output accumulator)
            # Rings are FIFO in emission order, so phase-1 and phase-2 are
            # emitted interleaved in true execution order: attention half h
            # right after head h's projections.
            with tc.tile_pool(name="ps", bufs=1, space="PSUM") as psp, \
                 tc.tile_pool(name="xin", bufs=3) as xin_pool, \
                 tc.tile_pool(name="xtp", bufs=1) as xtp, \
                 tc.tile_pool(name="wload", bufs=2) as wpool, \
                 tc.tile_pool(name="expp", bufs=3) as expp, \
                 tc.tile_pool(name="osp", bufs=2) as osp, \
                 tc.tile_pool(name="spp", bufs=1) as spp, \
                 tc.tile_pool(name="bcp", bufs=1) as bcp:

                wqf = [wpool.tile([128, 3 * H * D], bf16, tag=f"wqf{r}", name=f"wqf{r}", bufs=1) for r in range(4)]
                wkf = [wpool.tile([128, 2 * H * D], bf16, tag=f"wkf{r}", name=f"wkf{r}", bufs=1) for r in range(4)]
                wvf = [wpool.tile([128, H * D], bf16, tag=f"wvf{r}", name=f"wvf{r}", bufs=1) for r in range(4)]
                xtq = [xtp.tile([128, S], bf16, tag=f"xtq{c}", name=f"xtq{c}") for c in range(4)]
                xtk = [xtp.tile([128, S], bf16, tag=f"xtk{c}", name=f"xtk{c}") for c in range(4)]
                xtv = [xtp.tile([128, S], bf16, tag=f"xtv{c}", name=f"xtv{c}") for c in range(4)]

                GRP = min(4, NJ)
                NG = NJ // GRP

                def grp_block(x_dram, xt, g, nm):
                    for ii in range(GRP):
                        i = GRP * g + ii
                        xs = xin_pool.tile([128, DM], f32, tag="xs", name="xs")
                        nc.sync.dma_start(out=xs, in_=x_dram[128 * i:128 * (i + 1), :])
                        xq = xin_pool.tile([128, DM], bf16, tag="xin", name="xin")
                        nc.vector.tensor_copy(xq, xs)
                        tp = psp.tile([128, 4, 128], bf16, tag="proj", name="tp", bufs=2)
                        for c in range(4):
                            nc.tensor.transpose(
                                tp[:, c, :], xq[:, 128 * c:128 * (c + 1)], ident_b)
                        for c in range(4):
                            nc.vector.tensor_copy(
                                xt[c][:, 128 * i:128 * (i + 1)], tp[:, c, :])

                def proj_q(h, n):
                    ps = psp.tile([128, CH], f32, tag="proj", name="proj", bufs=2)
                    for r in range(4):
                        nc.tensor.matmul(
                            ps, wqf[r][:, 192 * h:192 * h + 128],
                            xtq[r][:, CH * n:CH * (n + 1)],
                            start=(r == 0), stop=(r == 3))
                    ts_(qp[h][:, CH * n:CH * (n + 1)], ps, bqp[:, h:h + 1],
                        None, OP.add)

                def proj_k(h, n):
                    ps = psp.tile([128, CH], f32, tag="proj", name="proj", bufs=2)
                    for r in range(4):
                        nc.tensor.matmul(
                            ps, wkf[r][:, 128 * h:128 * (h + 1)],
                            xtk[r][:, CH * n:CH * (n + 1)],
                            start=(r == 0), stop=(r == 3))
                    ts_(kp[h][:, CH * n:CH * (n + 1)], ps, bkp[:, h:h + 1],
                        None, OP.add)

                def proj_v(i):
                    ps = psp.tile([128, 512], f32, tag="proj", name="proj", bufs=2)
                    for r in range(4):
                        nc.tensor.matmul(
                            ps, xtv[r][:, 128 * i:128 * (i + 1)], wvf[r],
                            start=(r == 0), stop=(r == 3))
                    nc.vector.tensor_copy(
                        va[i][:, :, 0:64],
                        ps.rearrange("p (h d) -> p h d", d=64))
                    nc.gpsimd.memset(va[i][:, :, 64:65], 1.0)

                sp_h = [spp.tile([2, S], f32, tag=f"sph{i}", name=f"sph{i}")
                        for i in range(2)]
                rp_h = [spp.tile([2, S], f32, tag=f"rph{i}", name=f"rph{i}")
                        for i in range(2)]
                rscr = spp.tile([2, S], f32, tag="rscr", name="rscr")

                def attn_half(h):
                    p, half = divmod(h, 2)
                    sp = sp_h[half]
                    rp = rp_h[half]
                    oss = []
                    # term-sequential: scores K=64 (term t on PE row-half t)
                    for t in range(2):
                        rlo, rhi = 64 * t, 64 * t + 64
                        o_ps = psp.tile([65, S], f32, tag="o", name="o", bufs=1)
                        for j in range(NJ):
                            s_ps = psp.tile([128, S], f32, tag="s", name="s", bufs=2)
                            for n in range(NN):
                                nc.tensor.matmul(
                                    s_ps[:, CH * n:CH * (n + 1)],
                                    kp[h][rlo:rhi, 128 * j:128 * (j + 1)],
                                    qp[h][rlo:rhi, CH * n:CH * (n + 1)],
                                    start=True, stop=True)
                            ex = expp.tile([128, S], bf16, tag="exp", name="exp")
                            nc.scalar.activation(ex, s_ps, AF.Exp, scale=INV)
                            for n in range(NN):
                                nc.tensor.matmul(
                                    o_ps[:, CH * n:CH * (n + 1)],
                                    va[j][:, h, :],
                                    ex[:, CH * n:CH * (n + 1)],
                                    start=(j == 0), stop=(j == NJ - 1))
                        os_ = osp.tile([65, S], f32, tag=f"os{t}", name=f"os{t}")
                        nc.vector.tensor_copy(os_, o_ps)
                        nc.gpsimd.dma_start(
                            out=sp[t:t + 1, :], in_=os_[64:65, :])
                        oss.append(os_)
                    os1, os2 = oss
                    # per-half reciprocal of the two exp-sums
                    nc.vector.reciprocal_approx_accurate(rp, sp, rscr)

                    ydst = ypair[p][64 * half:64 * half + 64, :]
                    if h < 7:
                        # broadcast 1/sum across 64 partitions on Pool
                        r2 = bcp.tile([1, S], f32, tag="r2", name="r2")
                        nc.gpsimd.dma_start(out=r2, in_=rp[1:2, :])
                        bcs1 = bcp.tile([64, S], f32, tag="bcs1", name="bcs1")
                        nc.gpsimd.partition_broadcast(bcs1, rp[0:1, :], channels=64)
                        bcs2 = bcp.tile([64, S], f32, tag="bcs2", name="bcs2")
                        nc.gpsimd.partition_broadcast(bcs2, r2[0:1, :], channels=64)
                        nc.vector.tensor_mul(os1[0:64, :], os1[0:64, :], bcs1)
                        nc.vector.tensor_mul(os2[0:64, :], os2[0:64, :], bcs2)
                    else:
                        # exposed tail: selector matmul broadcast in PSUM
                        bc = psp.tile([128, S], f32, tag="s", name="bcpe", bufs=2)
                        for n in range(NN):
                            nc.tensor.matmul(
                                bc[:, CH * n:CH * (n + 1)], sel2,
                                rp[:, CH * n:CH * (n + 1)],
                                start=True, stop=True)
                        nc.vector.tensor_mul(
                            os1[0:64, :], os1[0:64, :], bc[0:64, :])
                        nc.vector.tensor_mul(
                            os2[0:64, :], os2[0:64, :], bc[64:128, :])
                    stt(ydst, os2[0:64, :], neglam64, os1[0:64, :],
                        OP.mult, OP.add, accum_out=sumcol[:, h:h + 1])
                    stt(os1[0:64, :], ydst, 1.0, ydst, OP.mult, OP.mult,
                        accum_out=sumcol[:, 8 + h:9 + h])

                # ---- emission: head-0 path first, attention interleaved ----
                for r in range(4):
                    ws0 = wpool.tile([128, 192], f32, tag="wsq0", name="wsq0")
                    nc.sync.dma_start(out=ws0, in_=wq_d[128 * r:128 * (r + 1), 0:192])
                    nc.scalar.copy(wqf[r][:, 0:192], ws0)
                for r in range(4):
                    ws0 = wpool.tile([128, 128], f32, tag="wsk0", name="wsk0")
                    nc.sync.dma_start(out=ws0, in_=wk_d[128 * r:128 * (r + 1), 0:128])
                    nc.scalar.copy(wkf[r][:, 0:128], ws0)
                grp_block(q_d, xtq, 0, "q")
                grp_block(k_d, xtk, 0, "k")
                proj_q(0, 0)
                proj_k(0, 0)
                wsv_s = []
                for r in range(4):
                    wsv = wpool.tile([128, H * D], f32, tag="wsv", name="wsv", bufs=2)
                    nc.sync.dma_start(out=wsv, in_=wv_d[128 * r:128 * (r + 1), :])
                    wsv_s.append(wsv)
                for g in range(1, NG):
                    grp_block(q_d, xtq, g, "q")
                    grp_block(k_d, xtk, g, "k")
                for r in range(4):
                    nc.vector.tensor_copy(wvf[r], wsv_s[r])
                for n in range(1, NN):
                    proj_q(0, n)
                    proj_k(0, n)
                for g in range(NG):
                    grp_block(v_d, xtv, g, "v")
                    for i in range(GRP * g, GRP * (g + 1)):
                        proj_v(i)
                # bulk weight columns: h1-3 slice first, then the rest
                for r in range(4):
                    wsq = wpool.tile([128, 576], f32, tag="wst", name="wsqA")
                    nc.sync.dma_start(out=wsq, in_=wq_d[128 * r:128 * (r + 1), 192:768])
                    nc.vector.tensor_copy(wqf[r][:, 192:768], wsq)
                    wsk = wpool.tile([128, 384], f32, tag="wst", name="wskA")
                    nc.sync.dma_start(out=wsk, in_=wk_d[128 * r:128 * (r + 1), 128:512])
                    nc.vector.tensor_copy(wkf[r][:, 128:512], wsk)
                for r in range(4):
                    wsq = wpool.tile([128, 3 * H * D - 768], f32, tag="wst", name="wsqB")
                    nc.sync.dma_start(out=wsq, in_=wq_d[128 * r:128 * (r + 1), 768:])
                    nc.vector.tensor_copy(wqf[r][:, 768:], wsq)
                    wsk = wpool.tile([128, 2 * H * D - 512], f32, tag="wst", name="wskB")
                    nc.sync.dma_start(out=wsk, in_=wk_d[128 * r:128 * (r + 1), 512:])
                    nc.vector.tensor_copy(wkf[r][:, 512:], wsk)

                attn_half(0)
                for h in range(1, 8):
                    for n in range(NN):
                        proj_q(h, n)
                        proj_k(h, n)
                    if h == 6:
                        # gate: pre-gathered pair-packed weight tiles
                        wgt = []
                        for r in range(4):
                            w_t = wpool.tile([128, 512], bf16, tag=f"wg{r}", name=f"wg{r}", bufs=1)
                            nc.gpsimd.dma_start(
                                out=w_t,
                                in_=wq_d[128 * r:128 * (r + 1), :].rearrange(
                                    "k (h blk) -> k h blk", blk=192)[:, :, 128:192])
                            wgt.append(w_t)
                        for p_ in range(4):
                            for n in range(NN):
                                ps = psp.tile([128, CH], f32, tag="proj", name="proj", bufs=2)
                                for r in range(4):
                                    nc.tensor.matmul(
                                        ps, wgt[r][:, 128 * p_:128 * (p_ + 1)],
                                        xtq[r][:, CH * n:CH * (n + 1)],
                                        start=(r == 0), stop=(r == 3))
                                ts_(gt[p_][:, CH * n:CH * (n + 1)], ps,
                                    bg[:, p_:p_ + 1], None, OP.add)
                    attn_half(h)

                # ---------- tail: gate tanh, stats, affine, output ----------
                with tc.tile_pool(name="tailp", bufs=1) as tailp, \
                     tc.tile_pool(name="oq", bufs=2) as oqp:

                    th_t = [osp.tile([128, S], f32, tag=f"os{p % 2}", name=f"th{p}")
                            for p in range(4)]
                    for p in range(4):
                        nc.scalar.activation(th_t[p], gt[p], AF.Tanh, scale=0.5)

                    tot = tailp.tile([64, 2], f32, tag="tot", name="tot")
                    nc.vector.tensor_reduce(
                        tot, sumcol.rearrange("p (t h) -> p t h", h=8),
                        axis=AX.X, op=OP.add)
                    csc = tailp.tile([64, 8], f32, tag="csc", name="csc")
                    nc.vector.tensor_mul(csc, cc, sumcol[:, 0:8])
                    cy64 = tailp.tile([64, 1], f32, tag="cy64", name="cy64")
                    nc.vector.tensor_reduce(cy64, csc, axis=AX.X, op=OP.add)
                    nc.vector.tensor_mul(csc, cc, cc)
                    csq64 = tailp.tile([64, 1], f32, tag="csq64", name="csq64")
                    nc.vector.tensor_reduce(csq64, csc, axis=AX.X, op=OP.add)
                    csum64 = tailp.tile([64, 1], f32, tag="csum64", name="csum64")
                    nc.vector.tensor_reduce(csum64, cc, axis=AX.X, op=OP.add)
                    tot2 = tailp.tile([64, 2], f32, tag="tot2", name="tot2")
                    stt(tot2[:, 0:1], csum64, float(S), tot[:, 0:1], OP.mult, OP.add)
                    stt(tot2[:, 1:2], cy64, 2.0, tot[:, 1:2], OP.mult, OP.add)
                    stt(tot2[:, 1:2], csq64, float(S), tot2[:, 1:2], OP.mult, OP.add)

                    ms_ps = psp.tile([64, 2], f32, tag="s", name="ms", bufs=2)
                    nc.tensor.matmul(ms_ps, ind2, tot2, start=True, stop=True)
                    mean64 = tailp.tile([64, 1], f32, tag="mean64", name="mean64")
                    ts_(mean64, ms_ps[:, 0:1], 1.0 / CNT, None, OP.mult)
                    e264 = tailp.tile([64, 1], f32, tag="e264", name="e264")
                    ts_(e264, ms_ps[:, 1:2], 1.0 / CNT, None, OP.mult)
                    nm2 = tailp.tile([64, 1], f32, tag="nm2", name="nm2")
                    ts_(nm2, mean64, mean64, -1.0, OP.mult, OP.mult)
                    veps = tailp.tile([64, 1], f32, tag="veps", name="veps")
                    stt(veps, nm2, EPS, e264, OP.add, OP.add)
                    sd = tailp.tile([64, 1], f32, tag="sd", name="sd")
                    nc.scalar.activation(sd, veps, AF.Sqrt)
                    rsd = tailp.tile([64, 1], f32, tag="rsd", name="rsd")
                    nc.vector.reciprocal(rsd, sd)
                    rr = tailp.tile([64, 1], f32, tag="rr", name="rr")
                    nc.vector.tensor_mul(rr, rsd, rsd)
                    nc.vector.tensor_mul(rr, rr, veps)
                    ts_(rr, rr, -0.5, 1.5, OP.mult, OP.add)
                    rstd = tailp.tile([64, 1], f32, tag="rstd", name="rstd")
                    nc.vector.tensor_mul(rstd, rsd, rr)

                    a64 = tailp.tile([64, 1], f32, tag="a64", name="a64")
                    ts_(a64, rstd, gamma_c, halfli, OP.mult, OP.mult)
                    cm = tailp.tile([64, 8], f32, tag="cm", name="cm")
                    ts_(cm, cc, mean64, None, OP.subtract)
                    ball = tailp.tile([64, 8], f32, tag="ball", name="ball")
                    ts_(ball, cm, a64, bb64, OP.mult, OP.add)

                    for p in range(4):
                        for half in range(2):
                            h = 2 * p + half
                            rows = ypair[p][64 * half:64 * half + 64, :]
                            ts_(rows, rows, a64, ball[:, h:h + 1], OP.mult, OP.add)
                        stt(ypair[p], th_t[p], 1.0, ypair[p], OP.add, OP.mult)

                    for c in range(NJ):
                        tp_o = psp.tile([128, 512], f32, tag="s", name="tp_out", bufs=2)
                        for p in range(4):
                            nc.tensor.transpose(
                                tp_o[:, 128 * p:128 * (p + 1)],
                                ypair[p][:, 128 * c:128 * (c + 1)], ident)
                        oq = oqp.tile([128, 512], f32, tag="oq", name="oq")
                        nc.scalar.copy(oq, tp_o)
                        nc.sync.dma_start(out=out_d[128 * c:128 * (c + 1), :], in_=oq)

    nc.finalize()
    return nc


_CACHE = {}


def _get_nc():
    if "nc" not in _CACHE:
        _CACHE["nc"] = build_nc(S_FULL)
    return _CACHE["nc"]


def run(inputs, trace=False, tmpdir=None):
    from concourse.bass_utils import run_bass_kernel_spmd
    nc = _get_nc()
    arrs = {k: np.asarray(v, dtype=np.float32) for k, v in inputs.items()}
    shared = {k: np.ascontiguousarray(arrs[k]) for k in
              ("Wq", "bq", "Wk", "bk", "Wv", "bv", "gamma", "beta",
               "lam", "lambda_init")}
    in_maps = []
    for i in range(B):
        m = dict(shared)
        m["query"] = np.ascontiguousarray(arrs["query"][i])
        m["key"] = np.ascontiguousarray(arrs["key"][i])
        m["values"] = np.ascontiguousarray(arrs["values"][i])
        in_maps.append(m)
    res = run_bass_kernel_spmd(nc, in_maps, core_ids=list(range(B)),
                               trace=trace, tmpdir=tmpdir)
    out = np.stack([res.results[i]["out"] for i in range(B)], axis=0)
    return out.astype(np.float32), res


def kernel(**inputs):
    out, _ = run(inputs)
    return out


# revision 23
# speedup vs baseline: 1.5958x; 1.0287x over previous
# Differential multi-head attention (dual softmax + GroupNorm + sigmoid gating)
# for Trainium2, batch-parallel across 8 NeuronCores (one batch row per core).
#
# Per-core math (batch b):
#   q = query @ Wq + bq -> per head: q1, q2, gate (each S x 64)
#   k = key   @ Wk + bk -> per head: k1, k2
#   v = values@ Wv + bv -> per head: v (S x 64)
#   attn = softmax(q1 k1^T / 8) - lam * softmax(q2 k2^T / 8)
#   out  = GroupNorm_{8 groups over d, reduced over (S, heads, d-in-group)}(attn @ v)
#   out  = out * (1 - lambda_init) * sigmoid(gate)
#
# Layout strategy: d-major ("transposed") attention: scores are computed as
# s^T (k on partitions, q free) so the attn@v contraction runs at K=128, and
# exp row-sums come free via a ones-column appended to v (M=65).
#
# Engine budget: ACT (scalar) is the bottleneck -- 128 exp tiles of
# [128,1024] at (N+352)/1.2GHz ~= 147us is the floor.  Everything else is
# kept off ACT: weight downcasts + x casts on Pool (gpsimd), projection
# epilogues + va/os copies + combines on DVE, gate tanh + output PSUM
# copies in the tail where ACT is idle.  Score matmuls are K=64 row-split
# (term1 rows 0-63, term2 rows 64-127, interleaved per chunk) so the PE
# streams both terms concurrently via row tiling.

import numpy as np

B, S_FULL, H, D = 8, 1024, 8, 64
DM = H * D  # 512


def build_nc(S=1024):
    import concourse.bacc as bacc
    import concourse.bass as bass
    import concourse.tile as tile
    from concourse import mybir
    from concourse.masks import make_identity

    f32 = mybir.dt.float32
    bf16 = mybir.dt.bfloat16
    AF = mybir.ActivationFunctionType
    OP = mybir.AluOpType
    AX = mybir.AxisListType

    NJ = S // 128          # k/seq 128-tiles
    CH = min(512, S)       # fp32-out matmul chunk
    NN = max(1, S // CH)
    CNT = float(S * H * (D // H))  # groupnorm reduction count per group
    EPS = 1e-3
    INV = 0.125            # 1/sqrt(64)

    nc = bacc.Bacc(target_bir_lowering=False)
    q_d = nc.dram_tensor("query", [S, DM], f32, kind="ExternalInput")
    k_d = nc.dram_tensor("key", [S, DM], f32, kind="ExternalInput")
    v_d = nc.dram_tensor("values", [S, DM], f32, kind="ExternalInput")
    wq_d = nc.dram_tensor("Wq", [DM, 3 * H * D], f32, kind="ExternalInput")
    bq_d = nc.dram_tensor("bq", [3 * H * D], f32, kind="ExternalInput")
    wk_d = nc.dram_tensor("Wk", [DM, 2 * H * D], f32, kind="ExternalInput")
    bk_d = nc.dram_tensor("bk", [2 * H * D], f32, kind="ExternalInput")
    wv_d = nc.dram_tensor("Wv", [DM, H * D], f32, kind="ExternalInput")
    bv_d = nc.dram_tensor("bv", [H * D], f32, kind="ExternalInput")
    gamma_d = nc.dram_tensor("gamma", [D], f32, kind="ExternalInput")
    beta_d = nc.dram_tensor("beta", [D], f32, kind="ExternalInput")
    lam_d = nc.dram_tensor("lam", [1], f32, kind="ExternalInput")
    li_d = nc.dram_tensor("lambda_init", [1], f32, kind="ExternalInput")
    out_d = nc.dram_tensor("out", [S, DM], f32, kind="ExternalOutput")

    ts_ = nc.vector.tensor_scalar
    stt = nc.vector.scalar_tensor_tensor

    with tile.TileContext(nc) as tc:
        with tc.tile_pool(name="consts", bufs=1) as consts, \
             tc.tile_pool(name="persist", bufs=1) as persist:

            # ---------- constants ----------
            # dummy exp to pull the ACT exp/tanh table load to t=0
            dmy = consts.tile([1, 8], f32, tag="dmy", name="dmy")
            nc.gpsimd.memset(dmy, 0.0)
            dmyo = consts.tile([1, 8], f32, tag="dmyo", name="dmyo")
            nc.scalar.activation(dmyo, dmy, AF.Exp)

            ident = consts.tile([128, 128], f32, tag="ident", name="ident")
            make_identity(nc, ident)
            ident_b = consts.tile([128, 128], bf16, tag="ident_b", name="ident_b")
            make_identity(nc, ident_b)

            # block-diagonal group matrix: IND2[d', d] = 1 iff d'//8 == d//8
            ind2 = consts.tile([64, 64], f32, tag="ind2", name="ind2")
            nc.gpsimd.memset(ind2, 1.0)
            nc.gpsimd.affine_select(
                out=ind2, in_=ind2, compare_op=OP.is_ge, fill=0.0,
                base=0, pattern=[[-8, 8], [0, 8]], channel_multiplier=1)
            nc.gpsimd.affine_select(
                out=ind2, in_=ind2, compare_op=OP.is_ge, fill=0.0,
                base=7, pattern=[[8, 8], [0, 8]], channel_multiplier=-1)

            # selector for the r-row broadcast matmul (used by the last pair)
            # sel2[p, x] = 1 iff (x - 64p) in [0, 64): row 0 -> parts 0-63,
            # row 1 -> parts 64-127
            sel2 = consts.tile([2, 128], f32, tag="sel2", name="sel2")
            nc.gpsimd.memset(sel2, 1.0)
            nc.gpsimd.affine_select(
                out=sel2, in_=sel2, compare_op=OP.is_ge, fill=0.0,
                base=0, pattern=[[1, 128]], channel_multiplier=-64)
            nc.gpsimd.affine_select(
                out=sel2, in_=sel2, compare_op=OP.is_ge, fill=0.0,
                base=63, pattern=[[-1, 128]], channel_multiplier=64)

            # scalar columns
            lam64 = consts.tile([64, 1], f32, tag="lam64", name="lam64")
            nc.gpsimd.dma_start(out=lam64, in_=lam_d[:].to_broadcast([64, 1]))
            li64 = consts.tile([64, 1], f32, tag="li64", name="li64")
            nc.gpsimd.dma_start(out=li64, in_=li_d[:].to_broadcast([64, 1]))
            neglam64 = consts.tile([64, 1], f32, tag="neglam64", name="neglam64")
            ts_(neglam64, lam64, -1.0, None, OP.mult)
            onelam64 = consts.tile([64, 1], f32, tag="onelam64", name="onelam64")
            ts_(onelam64, lam64, -1.0, 1.0, OP.mult, OP.add)   # 1 - lam
            halfli = consts.tile([64, 1], f32, tag="halfli", name="halfli")
            ts_(halfli, li64, -0.5, 0.5, OP.mult, OP.add)      # 0.5*(1-li)

            gamma_c = consts.tile([64, 1], f32, tag="gamma_c", name="gamma_c")
            nc.sync.dma_start(out=gamma_c, in_=gamma_d[:])
            beta_c = consts.tile([64, 1], f32, tag="beta_c", name="beta_c")
            nc.sync.dma_start(out=beta_c, in_=beta_d[:])
            bb64 = consts.tile([64, 1], f32, tag="bb64", name="bb64")
            ts_(bb64, beta_c, halfli, None, OP.mult)           # beta*0.5*(1-li)

            # v-bias columns per head and C = bv*(1-lam) fold
            bvc = consts.tile([64, 8], f32, tag="bvc", name="bvc")
            nc.sync.dma_start(
                out=bvc, in_=bv_d[:].rearrange("(h d) -> d h", d=64))
            cc = consts.tile([64, 8], f32, tag="cc", name="cc")
            ts_(cc, bvc, onelam64, None, OP.mult)

            # bias columns: per-head stacked [q1|q2] / [k1|k2] are contiguous
            # 128-element runs of bq/bk; gate needs a gathered layout.
            bqp = consts.tile([128, 8], f32, tag="bqp", name="bqp")
            nc.sync.dma_start(
                out=bqp,
                in_=bq_d[:].rearrange("(h blk) -> blk h", blk=192)[0:128, :])
            bkp = consts.tile([128, 8], f32, tag="bkp", name="bkp")
            nc.sync.dma_start(
                out=bkp,
                in_=bk_d[:].rearrange("(h blk) -> blk h", blk=128))
            bg = consts.tile([128, 4], f32, tag="bg", name="bg")
            bqv = bq_d[:].rearrange("(h blk) -> h blk", blk=192)
            for p in range(4):
                nc.sync.dma_start(out=bg[:, p:p + 1],
                                  in_=bqv[2 * p:2 * p + 2, 128:192])

            # persistent projection outputs (bf16, d-major)
            # qp/kp[h]: rows 0-63 = q1/k1 of head h, rows 64-127 = q2/k2
            qp = [persist.tile([128, S], bf16, tag=f"qp{h}", name=f"qp{h}") for h in range(8)]
            kp = [persist.tile([128, S], bf16, tag=f"kp{h}", name=f"kp{h}") for h in range(8)]
            # gate stays head-pair packed: gt[p] rows 0-63 = head 2p, 64-127 = 2p+1
            gt = [persist.tile([128, S], bf16, tag=f"gt{p}", name=f"gt{p}") for p in range(4)]
            va = [persist.tile([128, 8, 65], bf16, tag=f"va{i}", name=f"va{i}") for i in range(NJ)]
            ypair = [persist.tile([128, S], f32, tag=f"yp{p}", name=f"yp{p}") for p in range(4)]
            sumcol = persist.tile([64, 16], f32, tag="sumcol", name="sumcol")

            # ---------- unified PSUM pool: exactly 16KB of tag rings ----
            # tp   [128,512] bf16 x2 = 2KB   (input transposes)
            # proj [128,512] f32  x1 = 2KB   (projection accumulator)
            # s    [128,1024] f32 x2 = 8KB   (scores / bc / tail transposes)
            # o    [65,1024] f32  x1 = 4KB   (attn output accumulator)
            # Rings are FIFO in emission order, so phase-1 and phase-2 are
            # emitted interleaved in true execution order: attention half h
            # right after head h's projections.
            with tc.tile_pool(name="ps", bufs=1, space="PSUM") as psp, \
                 tc.tile_pool(name="xin", bufs=5) as xin_pool, \
                 tc.tile_pool(name="xtp", bufs=1) as xtp, \
                 tc.tile_pool(name="wload", bufs=2) as wpool, \
                 tc.tile_pool(name="expp", bufs=4) as expp, \
                 tc.tile_pool(name="osp", bufs=2) as osp, \
                 tc.tile_pool(name="spp", bufs=1) as spp, \
                 tc.tile_pool(name="bcp", bufs=1) as bcp:

                wqf = [wpool.tile([128, 3 * H * D], bf16, tag=f"wqf{r}", name=f"wqf{r}", bufs=1) for r in range(4)]
                wkf = [wpool.tile([128, 2 * H * D], bf16, tag=f"wkf{r}", name=f"wkf{r}", bufs=1) for r in range(4)]
                wvf = [wpool.tile([128, H * D], bf16, tag=f"wvf{r}", name=f"wvf{r}", bufs=1) for r in range(4)]
                xtq = [xtp.tile([128, S], bf16, tag=f"xtq{c}", name=f"xtq{c}") for c in range(4)]
                xtk = [xtp.tile([128, S], bf16, tag=f"xtk{c}", name=f"xtk{c}") for c in range(4)]
                xtv = [xtp.tile([128, S], bf16, tag=f"xtv{c}", name=f"xtv{c}") for c in range(4)]

                GRP = min(4, NJ)
                NG = NJ // GRP

                def grp_block(x_dram, xt, g, nm, on_act=False):
                    for ii in range(GRP):
                        i = GRP * g + ii
                        xs = xin_pool.tile([128, DM], f32, tag="xs", name="xs")
                        nc.sync.dma_start(out=xs, in_=x_dram[128 * i:128 * (i + 1), :])
                        xq = xin_pool.tile([128, DM], bf16, tag="xin", name="xin")
                        nc.vector.tensor_copy(xq, xs)
                        tp = psp.tile([128, 4, 128], bf16, tag="proj", name="tp", bufs=2)
                        for c in range(4):
                            nc.tensor.transpose(
                                tp[:, c, :], xq[:, 128 * c:128 * (c + 1)], ident_b)
                        if on_act:
                            # ACT is idle pre-attention; use it for the
                            # PSUM->SBUF moves to unclog the DVE lead-in
                            nc.scalar.copy(
                                xt[0][:, 128 * i:128 * (i + 1)], tp[:, 0, :])
                            nc.scalar.copy(
                                xt[1][:, 128 * i:128 * (i + 1)], tp[:, 1, :])
                            nc.vector.tensor_copy(
                                xt[2][:, 128 * i:128 * (i + 1)], tp[:, 2, :])
                            nc.vector.tensor_copy(
                                xt[3][:, 128 * i:128 * (i + 1)], tp[:, 3, :])
                        else:
                            for c in range(4):
                                nc.vector.tensor_copy(
                                    xt[c][:, 128 * i:128 * (i + 1)], tp[:, c, :])

                def proj_q(h, n):
                    ps = psp.tile([128, CH], f32, tag="proj", name="proj", bufs=2)
                    for r in range(4):
                        nc.tensor.matmul(
                            ps, wqf[r][:, 192 * h:192 * h + 128],
                            xtq[r][:, CH * n:CH * (n + 1)],
                            start=(r == 0), stop=(r == 3))
                    ts_(qp[h][:, CH * n:CH * (n + 1)], ps, bqp[:, h:h + 1],
                        None, OP.add)

                def proj_k(h, n):
                    ps = psp.tile([128, CH], f32, tag="proj", name="proj", bufs=2)
                    for r in range(4):
                        nc.tensor.matmul(
                            ps, wkf[r][:, 128 * h:128 * (h + 1)],
                            xtk[r][:, CH * n:CH * (n + 1)],
                            start=(r == 0), stop=(r == 3))
                    ts_(kp[h][:, CH * n:CH * (n + 1)], ps, bkp[:, h:h + 1],
                        None, OP.add)

                def proj_v(i):
                    ps = psp.tile([128, 512], f32, tag="proj", name="proj", bufs=2)
                    for r in range(4):
                        nc.tensor.matmul(
                            ps, xtv[r][:, 128 * i:128 * (i + 1)], wvf[r],
                            start=(r == 0), stop=(r == 3))
                    nc.scalar.copy(
                        va[i][:, :, 0:64],
                        ps.rearrange("p (h d) -> p h d", d=64))
                    nc.gpsimd.memset(va[i][:, :, 64:65], 1.0)

                sp_h = [spp.tile([2, S], f32, tag=f"sph{i}", name=f"sph{i}")
                        for i in range(2)]
                rp_h = [spp.tile([2, S], f32, tag=f"rph{i}", name=f"rph{i}")
                        for i in range(2)]
                rscr = spp.tile([2, S], f32, tag="rscr", name="rscr")

                def attn_half(h):
                    p, half = divmod(h, 2)
                    sp = sp_h[half]
                    rp = rp_h[half]
                    oss = []
                    # term-sequential: scores K=64 (term t on PE row-half t)
                    for t in range(2):
                        rlo, rhi = 64 * t, 64 * t + 64
                        o_ps = psp.tile([65, S], f32, tag="o", name="o", bufs=1)
                        for j in range(NJ):
                            s_ps = psp.tile([128, S], f32, tag="s", name="s", bufs=2)
                            for n in range(NN):
                                nc.tensor.matmul(
                                    s_ps[:, CH * n:CH * (n + 1)],
                                    kp[h][rlo:rhi, 128 * j:128 * (j + 1)],
                                    qp[h][rlo:rhi, CH * n:CH * (n + 1)],
                                    start=True, stop=True)
                            ex = expp.tile([128, S], bf16, tag="exp", name="exp")
                            nc.scalar.activation(ex, s_ps, AF.Exp, scale=INV)
                            for n in range(NN):
                                nc.tensor.matmul(
                                    o_ps[:, CH * n:CH * (n + 1)],
                                    va[j][:, h, :],
                                    ex[:, CH * n:CH * (n + 1)],
                                    start=(j == 0), stop=(j == NJ - 1))
                        os_ = osp.tile([65, S], f32, tag=f"os{t}", name=f"os{t}")
                        nc.vector.tensor_copy(os_, o_ps)
                        nc.sync.dma_start(
                            out=sp[t:t + 1, :], in_=os_[64:65, :])
                        oss.append(os_)
                    os1, os2 = oss
                    # per-half reciprocal of the two exp-sums
                    nc.vector.reciprocal_approx_accurate(rp, sp, rscr)

                    ydst = ypair[p][64 * half:64 * half + 64, :]
                    if h < 7:
                        # broadcast 1/sum across 64 partitions on Pool
                        r2 = bcp.tile([1, S], f32, tag="r2", name="r2")
                        nc.gpsimd.dma_start(out=r2, in_=rp[1:2, :])
                        bcs1 = bcp.tile([64, S], f32, tag="bcs1", name="bcs1")
                        nc.gpsimd.partition_broadcast(bcs1, rp[0:1, :], channels=64)
                        bcs2 = bcp.tile([64, S], f32, tag="bcs2", name="bcs2")
                        nc.gpsimd.partition_broadcast(bcs2, r2[0:1, :], channels=64)
                        nc.vector.tensor_mul(os1[0:64, :], os1[0:64, :], bcs1)
                        nc.vector.tensor_mul(os2[0:64, :], os2[0:64, :], bcs2)
                    else:
                        # exposed tail: keep the whole chain on DVE
                        r2 = bcp.tile([1, S], f32, tag="r2", name="r2")
                        nc.sync.dma_start(out=r2, in_=rp[1:2, :])
                        bcs1 = bcp.tile([64, S], f32, tag="bcs1", name="bcs1")
                        nc.gpsimd.partition_broadcast(bcs1, rp[0:1, :], channels=64)
                        nc.vector.tensor_mul(os1[0:64, :], os1[0:64, :], bcs1)
                        bcs2 = bcp.tile([64, S], f32, tag="bcs2", name="bcs2")
                        nc.gpsimd.partition_broadcast(bcs2, r2[0:1, :], channels=64)
                        nc.vector.tensor_mul(os2[0:64, :], os2[0:64, :], bcs2)
                    stt(ydst, os2[0:64, :], neglam64, os1[0:64, :],
                        OP.mult, OP.add, accum_out=sumcol[:, h:h + 1])
                    stt(os1[0:64, :], ydst, 1.0, ydst, OP.mult, OP.mult,
                        accum_out=sumcol[:, 8 + h:9 + h])

                # ---- emission: head-0 path first, attention interleaved ----
                for r in range(4):
                    ws0 = wpool.tile([128, 192], f32, tag="wsq0", name="wsq0")
                    nc.sync.dma_start(out=ws0, in_=wq_d[128 * r:128 * (r + 1), 0:192])
                    nc.scalar.copy(wqf[r][:, 0:192], ws0)
                for r in range(4):
                    ws0 = wpool.tile([128, 128], f32, tag="wsk0", name="wsk0")
                    nc.sync.dma_start(out=ws0, in_=wk_d[128 * r:128 * (r + 1), 0:128])
                    nc.scalar.copy(wkf[r][:, 0:128], ws0)
                grp_block(q_d, xtq, 0, "q", on_act=True)
                grp_block(k_d, xtk, 0, "k", on_act=True)
                wsv_s = []
                for r in range(4):
                    wsv = wpool.tile([128, H * D], f32, tag="wsv", name="wsv", bufs=2)
                    nc.sync.dma_start(out=wsv, in_=wv_d[128 * r:128 * (r + 1), :])
                    wsv_s.append(wsv)
                grp_block(v_d, xtv, 0, "v", on_act=True)
                proj_q(0, 0)
                proj_k(0, 0)
                grp_block(q_d, xtq, 1, "q")
                grp_block(k_d, xtk, 1, "k")
                for r in range(4):
                    nc.vector.tensor_copy(wvf[r], wsv_s[r])
                for i in range(GRP):
                    proj_v(i)
                for n in range(1, NN):
                    proj_q(0, n)
                    proj_k(0, n)
                grp_block(v_d, xtv, 1, "v")
                for i in range(GRP, 2 * GRP):
                    proj_v(i)
                # bulk weight columns: h1-3 slice before attention, rest after
                for r in range(4):
                    wsq = wpool.tile([128, 576], f32, tag="wst", name="wsqA")
                    nc.sync.dma_start(out=wsq, in_=wq_d[128 * r:128 * (r + 1), 192:768])
                    nc.vector.tensor_copy(wqf[r][:, 192:768], wsq)
                    wsk = wpool.tile([128, 384], f32, tag="wst", name="wskA")
                    nc.sync.dma_start(out=wsk, in_=wk_d[128 * r:128 * (r + 1), 128:512])
                    nc.vector.tensor_copy(wkf[r][:, 128:512], wsk)

                attn_half(0)
                for r in range(4):
                    wsq = wpool.tile([128, 3 * H * D - 768], f32, tag="wst", name="wsqB")
                    nc.sync.dma_start(out=wsq, in_=wq_d[128 * r:128 * (r + 1), 768:])
                    nc.vector.tensor_copy(wqf[r][:, 768:], wsq)
                    wsk = wpool.tile([128, 2 * H * D - 512], f32, tag="wst", name="wskB")
                    nc.sync.dma_start(out=wsk, in_=wk_d[128 * r:128 * (r + 1), 512:])
                    nc.vector.tensor_copy(wkf[r][:, 512:], wsk)
                for h in range(1, 8):
                    for n in range(NN):
                        proj_q(h, n)
                        proj_k(h, n)
                    if h == 6:
                        # gate: pre-gathered pair-packed weight tiles
                        wgt = []
                        for r in range(4):
                            w_t = wpool.tile([128, 512], bf16, tag=f"wg{r}", name=f"wg{r}", bufs=1)
                            nc.gpsimd.dma_start(
                                out=w_t,
                                in_=wq_d[128 * r:128 * (r + 1), :].rearrange(
                                    "k (h blk) -> k h blk", blk=192)[:, :, 128:192])
                            wgt.append(w_t)
                        for p_ in range(4):
                            for n in range(NN):
                                ps = psp.tile([128, CH], f32, tag="proj", name="proj", bufs=2)
                                for r in range(4):
                                    nc.tensor.matmul(
                                        ps, wgt[r][:, 128 * p_:128 * (p_ + 1)],
                                        xtq[r][:, CH * n:CH * (n + 1)],
                                        start=(r == 0), stop=(r == 3))
                                ts_(gt[p_][:, CH * n:CH * (n + 1)], ps,
                                    bg[:, p_:p_ + 1], None, OP.add)
                    attn_half(h)

                # ---------- tail: gate tanh, stats, affine, output ----------
                with tc.tile_pool(name="tailp", bufs=1) as tailp, \
                     tc.tile_pool(name="oq", bufs=2) as oqp:

                    th_t = [osp.tile([128, S], f32, tag=f"os{p % 2}", name=f"th{p}")
                            for p in range(4)]
                    for p in range(4):
                        nc.scalar.activation(th_t[p], gt[p], AF.Tanh, scale=0.5)

                    tot = tailp.tile([64, 2], f32, tag="tot", name="tot")
                    nc.vector.tensor_reduce(
                        tot, sumcol.rearrange("p (t h) -> p t h", h=8),
                        axis=AX.X, op=OP.add)
                    csc = tailp.tile([64, 8], f32, tag="csc", name="csc")
                    nc.vector.tensor_mul(csc, cc, sumcol[:, 0:8])
                    cy64 = tailp.tile([64, 1], f32, tag="cy64", name="cy64")
                    nc.vector.tensor_reduce(cy64, csc, axis=AX.X, op=OP.add)
                    nc.vector.tensor_mul(csc, cc, cc)
                    csq64 = tailp.tile([64, 1], f32, tag="csq64", name="csq64")
                    nc.vector.tensor_reduce(csq64, csc, axis=AX.X, op=OP.add)
                    csum64 = tailp.tile([64, 1], f32, tag="csum64", name="csum64")
                    nc.vector.tensor_reduce(csum64, cc, axis=AX.X, op=OP.add)
                    tot2 = tailp.tile([64, 2], f32, tag="tot2", name="tot2")
                    stt(tot2[:, 0:1], csum64, float(S), tot[:, 0:1], OP.mult, OP.add)
                    stt(tot2[:, 1:2], cy64, 2.0, tot[:, 1:2], OP.mult, OP.add)
                    stt(tot2[:, 1:2], csq64, float(S), tot2[:, 1:2], OP.mult, OP.add)

                    ms_ps = psp.tile([64, 2], f32, tag="s", name="ms", bufs=2)
                    nc.tensor.matmul(ms_ps, ind2, tot2, start=True, stop=True)
                    mean64 = tailp.tile([64, 1], f32, tag="mean64", name="mean64")
                    ts_(mean64, ms_ps[:, 0:1], 1.0 / CNT, None, OP.mult)
                    e264 = tailp.tile([64, 1], f32, tag="e264", name="e264")
                    ts_(e264, ms_ps[:, 1:2], 1.0 / CNT, None, OP.mult)
                    nm2 = tailp.tile([64, 1], f32, tag="nm2", name="nm2")
                    ts_(nm2, mean64, mean64, -1.0, OP.mult, OP.mult)
                    veps = tailp.tile([64, 1], f32, tag="veps", name="veps")
                    stt(veps, nm2, EPS, e264, OP.add, OP.add)
                    sd = tailp.tile([64, 1], f32, tag="sd", name="sd")
                    nc.scalar.activation(sd, veps, AF.Sqrt)
                    rsd = tailp.tile([64, 1], f32, tag="rsd", name="rsd")
                    nc.vector.reciprocal(rsd, sd)
                    rr = tailp.tile([64, 1], f32, tag="rr", name="rr")
                    nc.vector.tensor_mul(rr, rsd, rsd)
                    nc.vector.tensor_mul(rr, rr, veps)
                    ts_(rr, rr, -0.5, 1.5, OP.mult, OP.add)
                    rstd = tailp.tile([64, 1], f32, tag="rstd", name="rstd")
                    nc.vector.tensor_mul(rstd, rsd, rr)

                    a64 = tailp.tile([64, 1], f32, tag="a64", name="a64")
                    ts_(a64, rstd, gamma_c, halfli, OP.mult, OP.mult)
                    cm = tailp.tile([64, 8], f32, tag="cm", name="cm")
                    ts_(cm, cc, mean64, None, OP.subtract)
                    ball = tailp.tile([64, 8], f32, tag="ball", name="ball")
                    ts_(ball, cm, a64, bb64, OP.mult, OP.add)

                    for p in range(4):
                        for half in range(2):
                            h = 2 * p + half
                            rows = ypair[p][64 * half:64 * half + 64, :]
                            ts_(rows, rows, a64, ball[:, h:h + 1], OP.mult, OP.add)
                        # gated output in bf16, reusing qp[p] (dead after the
                        # last scores) -> 1-cycle/row output transposes
                        stt(qp[p], th_t[p], 1.0, ypair[p], OP.add, OP.mult)

                    for c in range(NJ):
                        tp_o = psp.tile([128, 512], bf16, tag="s", name="tp_out", bufs=2)
                        for p in range(4):
                            nc.tensor.transpose(
                                tp_o[:, 128 * p:128 * (p + 1)],
                                qp[p][:, 128 * c:128 * (c + 1)], ident_b)
                        oq = oqp.tile([128, 512], f32, tag="oq", name="oq")
                        nc.scalar.copy(oq, tp_o)
                        nc.sync.dma_start(out=out_d[128 * c:128 * (c + 1), :], in_=oq)

    nc.finalize()
    return nc


_CACHE = {}


def _get_nc():
    if "nc" not in _CACHE:
        _CACHE["nc"] = build_nc(S_FULL)
    return _CACHE["nc"]


def run(inputs, trace=False, tmpdir=None):
    from concourse.bass_utils import run_bass_kernel_spmd
    nc = _get_nc()
    arrs = {k: np.asarray(v, dtype=np.float32) for k, v in inputs.items()}
    shared = {k: np.ascontiguousarray(arrs[k]) for k in
              ("Wq", "bq", "Wk", "bk", "Wv", "bv", "gamma", "beta",
               "lam", "lambda_init")}
    in_maps = []
    for i in range(B):
        m = dict(shared)
        m["query"] = np.ascontiguousarray(arrs["query"][i])
        m["key"] = np.ascontiguousarray(arrs["key"][i])
        m["values"] = np.ascontiguousarray(arrs["values"][i])
        in_maps.append(m)
    res = run_bass_kernel_spmd(nc, in_maps, core_ids=list(range(B)),
                               trace=trace, tmpdir=tmpdir)
    out = np.stack([res.results[i]["out"] for i in range(B)], axis=0)
    return out.astype(np.float32), res


def kernel(**inputs):
    out, _ = run(inputs)
    return out
